# revision 1
# baseline (speedup 1.0000x reference)
"""Trainium2 Bass kernel for BioBERT-ARG-GNN (gated pooling + 2-layer GCN + MLP head).

Strategy: pure data parallel over batch B=64 across 8 NeuronCores (8 graphs
per core).  All segment/gather ops are dense matmuls against one-hot
matrices built on-device from the index tensors (N=128 nodes == partition
dim).  GCN normalization (D^-1/2 (A+I) D^-1/2) factors into per-partition
scalings around a dense [128,128] adjacency matmul.  Matmul dtypes: f32r
(TF32-like, 1 cycle/row at free-dim>=256) for the big subtoken pooling,
bf16 for the [128,128] GCN matmuls (adjacency counts are exact), f32 for
the tiny FC head.  Phase split keeps each ACT function's table loaded once.
"""

import os
import sys

import numpy as np

for _p in ("/opt/trn_rl_repo", "/root/.axon_site/_ro/trn_rl_repo"):
    if os.path.isdir(_p) and _p not in sys.path:
        sys.path.insert(0, _p)

import ml_dtypes  # noqa: E402
import concourse.bass as bass  # noqa: E402
import concourse.mybir as mybir  # noqa: E402
from concourse import tile  # noqa: E402
from concourse.bass_utils import run_bass_kernel_spmd  # noqa: E402

# Problem shapes (hardcoded per contest rules).
B, S, H = 64, 512, 768
N, E = 128, 1024
GH, FH, L = 128, 256, 2
NCORES = 8
BL = B // NCORES  # graphs per core
SC = S // 128     # subtoken chunks per graph
EC = E // 128     # edge chunks per graph
HC = H // 128     # BERT-hidden chunks
FC = (H + GH) // 128  # concat-feature chunks for the FC head

f32 = mybir.dt.float32
f32r = mybir.dt.float32r
bf16 = mybir.dt.bfloat16
AFT = mybir.ActivationFunctionType
ALU = mybir.AluOpType
BF16 = ml_dtypes.bfloat16

_CACHE = {}


def _split_multi_waits(nc: bass.Bass) -> int:
    """Walrus in this container accepts one sync-wait per instruction; split
    extra waits into single-wait EventSemaphore nops just before it."""
    n_split = 0
    for fn in nc.m.functions:
        for blk in fn.blocks:
            new_instrs = []
            changed = False
            for inst in blk.instructions:
                si = getattr(inst, "sync_info", None)
                if si is not None and si.on_wait is not None and len(si.on_wait) > 1:
                    waits = list(si.on_wait)
                    for j, w in enumerate(waits[:-1]):
                        ev = mybir.InstEventSemaphore(
                            name=f"{inst.name}_ws{j}",
                            ins=[], outs=[],
                            engine=inst.engine,
                            sync_info=mybir.SyncInfo(on_wait=[w], on_update=[]),
                        )
                        new_instrs.append(ev)
                    inst.sync_info = mybir.SyncInfo(
                        on_wait=[waits[-1]], on_update=list(si.on_update))
                    n_split += 1
                    changed = True
                new_instrs.append(inst)
            if changed:
                blk.instructions = new_instrs
    return n_split


def build_program(br_val: float, b1_zero: bool, b2_zero: bool) -> bass.Bass:
    nc = bass.Bass()

    lh_d = nc.declare_dram_parameter("lh", [BL, S, H], f32r, isOutput=False)
    subv_d = nc.declare_dram_parameter("subv", [BL, 128, SC], f32, isOutput=False)
    esrc_d = nc.declare_dram_parameter("esrc", [BL, 128, EC], f32, isOutput=False)
    edst_d = nc.declare_dram_parameter("edst", [BL, 128, EC], f32, isOutput=False)
    wrb_d = nc.declare_dram_parameter("wrb", [128, H], f32, isOutput=False)
    w1t_d = nc.declare_dram_parameter("w1t", [128, HC, GH], bf16, isOutput=False)
    w2t_d = nc.declare_dram_parameter("w2t", [GH, GH], bf16, isOutput=False)
    wf1t_d = nc.declare_dram_parameter("wf1t", [128, FC, FH], f32, isOutput=False)
    wf2t_d = nc.declare_dram_parameter("wf2t", [128, 2, L], f32, isOutput=False)
    b1b_d = nc.declare_dram_parameter("b1b", [128, GH], f32, isOutput=False)
    b2b_d = nc.declare_dram_parameter("b2b", [128, GH], f32, isOutput=False)
    bf1b_d = nc.declare_dram_parameter("bf1b", [BL, FH], f32, isOutput=False)
    bf2b_d = nc.declare_dram_parameter("bf2b", [BL, L], f32, isOutput=False)
    iotaf_d = nc.declare_dram_parameter("iota_f", [128, 128], f32, isOutput=False)
    iota8_d = nc.declare_dram_parameter("iota8", [128, EC, 128], f32, isOutput=False)
    identb_d = nc.declare_dram_parameter("ident_b", [128, 128], bf16, isOutput=False)
    identf_d = nc.declare_dram_parameter("ident_f", [128, 128], f32, isOutput=False)
    onesr_d = nc.declare_dram_parameter("ones_r", [128, 1], f32r, isOutput=False)
    onesb_d = nc.declare_dram_parameter("ones_b", [128, 1], bf16, isOutput=False)
    meanb_d = nc.declare_dram_parameter("mean_b", [128, 1], bf16, isOutput=False)
    out_d = nc.declare_dram_parameter("out", [BL, L], f32, isOutput=True)

    with tile.TileContext(nc) as tc:
        with (
            tc.tile_pool(name="const", bufs=1) as cpool,
            tc.tile_pool(name="lhp", bufs=8) as lhpool,
            tc.tile_pool(name="scr", bufs=3) as scpool,
            tc.tile_pool(name="work", bufs=3) as wpool,
            tc.tile_pool(name="small", bufs=6) as spool,
            tc.tile_pool(name="psA", bufs=2, space="PSUM") as psA,
            tc.tile_pool(name="psB", bufs=2, space="PSUM") as psB,
            tc.tile_pool(name="psC", bufs=2, space="PSUM") as psC,
        ):
            # ---- early constants (ACT HWDGE ring; SP ring is reserved for lh) ----
            iota8 = cpool.tile([128, EC, 128], f32)
            nc.scalar.dma_start(iota8[:], iota8_d[:])
            ident_b = cpool.tile([128, 128], bf16)
            nc.scalar.dma_start(ident_b[:], identb_d[:])
            ones_b = cpool.tile([128, 1], bf16)
            nc.scalar.dma_start(ones_b[:], onesb_d[:])
            wrb = cpool.tile([128, H], f32)
            nc.scalar.dma_start(wrb[:], wrb_d[:])
            w1s = cpool.tile([128, HC, GH], bf16)
            nc.scalar.dma_start(w1s[:], w1t_d[:])
            w2s = cpool.tile([GH, GH], bf16)
            nc.scalar.dma_start(w2s[:], w2t_d[:])
            mean_b = cpool.tile([128, 1], bf16)
            nc.scalar.dma_start(mean_b[:], meanb_d[:])
            # pooled graph embeddings (written one column per graph)
            catT6 = cpool.tile([128, BL], f32)

            # ---------- phase 0: adjacency + degrees for all graphs ----------
            atis = []
            dinvs = []
            subvs = []
            for g in range(BL):
                subv = spool.tile([128, SC], f32, tag="subv", bufs=BL)
                nc.sync.dma_start(subv[:], subv_d[g])
                subvs.append(subv)
                esrc = spool.tile([128, EC], f32, tag="esrc", bufs=2)
                nc.sync.dma_start(esrc[:], esrc_d[g])
                edst = spool.tile([128, EC], f32, tag="edst", bufs=2)
                nc.sync.dma_start(edst[:], edst_d[g])

                at_ps = psB.tile([128, 128], f32, tag="mm")
                s_all = wpool.tile([128, EC, 128], bf16, tag="ohS")
                nc.vector.tensor_tensor(
                    out=s_all[:], in0=iota8[:],
                    in1=esrc[:].broadcast_to([128, EC, 128]), op=ALU.is_equal)
                d_all = wpool.tile([128, EC, 128], bf16, tag="ohD")
                nc.vector.tensor_tensor(
                    out=d_all[:], in0=iota8[:],
                    in1=edst[:].broadcast_to([128, EC, 128]), op=ALU.is_equal)
                for c in range(EC):
                    nc.tensor.matmul(at_ps[:], s_all[:, c, :], d_all[:, c, :],
                                     start=(c == 0), stop=False)
                # += I (self-loops) via identity outer product, exact in bf16
                nc.tensor.matmul(at_ps[:], ident_b[:], ident_b[:], start=False,
                                 stop=True)
                ati = wpool.tile([128, 128], bf16, tag="ati", bufs=BL)
                nc.scalar.copy(ati[:], at_ps[:])
                atis.append(ati)
                # deg[d] = sum_s ATI[s,d]  -> dinv = 1/sqrt(deg)
                deg_ps = psB.tile([128, 1], f32, tag="mm")
                nc.tensor.matmul(deg_ps[:], ati[:], ones_b[:],
                                 start=True, stop=True)
                sdeg = spool.tile([128, 1], f32, tag="sv")
                nc.scalar.activation(sdeg[:], deg_ps[:], AFT.Sqrt)
                dinv = spool.tile([128, 1], f32, tag="dinv", bufs=BL)
                nc.vector.reciprocal(dinv[:], sdeg[:])
                dinvs.append(dinv)

            # ---------- phase 1: gate + pooling + GCN per graph ----------
            _b1b = [None]
            _b2b = [None]
            for g in range(BL):
                subv = subvs[g]
                ati = atis[g]
                dinv = dinvs[g]

                cnt_ps = psC.tile([128, SC], f32, tag="cnt")
                nf_ps = psA.tile([128, H], f32, tag="nf")
                p_all = wpool.tile([128, SC, 128], bf16, tag="ohP")
                nc.vector.tensor_tensor(
                    out=p_all[:], in0=iota8[:, 0:SC, :],
                    in1=subv[:].broadcast_to([128, SC, 128]), op=ALU.is_equal)
                for c in range(SC):
                    lht = lhpool.tile([128, H], f32r, tag="lh")
                    nc.sync.dma_start(lht[:], lh_d[g, c * 128 : (c + 1) * 128, :])
                    scr = scpool.tile([128, H], bf16, tag="scr")
                    logits = spool.tile([128, 1], f32, tag="sv")
                    nc.vector.scalar_tensor_tensor(
                        scr[:], lht[:].bitcast(f32), 0.0, wrb[:], ALU.bypass,
                        ALU.mult, accum_out=logits[:])
                    gate = spool.tile([128, 1], f32, tag="sv")
                    nc.scalar.activation(gate[:], logits[:], AFT.Sigmoid,
                                         bias=float(br_val))
                    pg_t = wpool.tile([128, 128], f32r, tag="ohPg")
                    nc.scalar.mul(pg_t[:], p_all[:, c, :], gate[:])
                    nc.tensor.matmul(cnt_ps[:, c : c + 1], p_all[:, c, :],
                                     ones_b[:], start=True, stop=True)
                    # pooled node feats: nf[n,h] += Pg[s,n]^T lh[s,h]
                    nc.tensor.matmul(nf_ps[:, 0:512], pg_t[:], lht[:, 0:512],
                                     start=(c == 0), stop=(c == SC - 1))
                    nc.tensor.matmul(nf_ps[:, 512:H], pg_t[:], lht[:, 512:H],
                                     start=(c == 0), stop=(c == SC - 1))

                # 1/max(cnt,1); combined layer-1 row scale s1 = invc * dinv
                cnt1 = spool.tile([128, 1], f32, tag="sv")
                nc.vector.tensor_reduce(cnt1[:], cnt_ps[:], mybir.AxisListType.X,
                                        ALU.add)
                mx = spool.tile([128, 1], f32, tag="sv")
                nc.vector.tensor_scalar_max(mx[:], cnt1[:], 1.0)
                invc = spool.tile([128, 1], f32, tag="sv")
                nc.vector.reciprocal(invc[:], mx[:])
                s1 = spool.tile([128, 1], f32, tag="sv")
                nc.vector.tensor_tensor(s1[:], invc[:], dinv[:], ALU.mult)

                # scale rows by s1 while moving PSUM->SBUF (bf16 for layer 1)
                nf_sb = wpool.tile([128, H], bf16, tag="nfsb", bufs=2)
                nc.vector.tensor_scalar_mul(nf_sb[:], nf_ps[:], s1[:])
                # transpose to nfT chunks [h,n]
                nfs = wpool.tile([128, HC, GH], bf16, tag="nfs", bufs=2)
                for hc in range(HC):
                    tr_ps = psB.tile([128, 128], bf16, tag="mm")
                    nc.tensor.transpose(tr_ps[:], nf_sb[:, hc * 128 : (hc + 1) * 128],
                                        ident_b[:])
                    nc.any.tensor_copy(nfs[:, hc, :], tr_ps[:])

                # GCN layer 1: T2 = (s1*sums) @ W1  (scale pre-applied)
                t1_ps = psB.tile([128, GH], f32, tag="mm")
                for hc in range(HC):
                    nc.tensor.matmul(t1_ps[:], nfs[:, hc, :], w1s[:, hc, :],
                                     start=(hc == 0), stop=(hc == HC - 1))
                t2 = wpool.tile([128, GH], bf16, tag="t2")
                nc.any.tensor_copy(t2[:], t1_ps[:])
                z_ps = psB.tile([128, GH], f32, tag="mm")
                nc.tensor.matmul(z_ps[:], ati[:], t2[:], start=True, stop=True)
                x1 = wpool.tile([128, GH], bf16, tag="x1")
                if b1_zero:
                    # x1 = dinv * relu(z)  (valid since dinv > 0)
                    nc.vector.tensor_scalar(x1[:], z_ps[:], 0.0, dinv[:],
                                            ALU.max, ALU.mult)
                else:
                    if g == 0 and _b1b[0] is None:
                        _b1b[0] = cpool.tile([128, GH], f32, name="b1bt")
                        nc.scalar.dma_start(_b1b[0][:], b1b_d[:])
                    x1p = wpool.tile([128, GH], f32, tag="x1p")
                    nc.vector.scalar_tensor_tensor(x1p[:], z_ps[:], dinv[:],
                                                   _b1b[0][:], ALU.mult, ALU.add)
                    nc.vector.tensor_scalar_max(x1[:], x1p[:], 0.0)

                # GCN layer 2
                x1t_ps = psB.tile([128, GH], bf16, tag="mm")
                nc.tensor.transpose(x1t_ps[:], x1[:], ident_b[:])
                x1t = wpool.tile([128, GH], bf16, tag="x1t")
                nc.any.tensor_copy(x1t[:], x1t_ps[:])
                tp_ps = psB.tile([128, GH], f32, tag="mm")
                nc.tensor.matmul(tp_ps[:], x1t[:], w2s[:], start=True, stop=True)
                t2p = wpool.tile([128, GH], bf16, tag="t2")
                nc.vector.tensor_scalar_mul(t2p[:], tp_ps[:], dinv[:])
                z2_ps = psB.tile([128, GH], f32, tag="mm")
                nc.tensor.matmul(z2_ps[:], ati[:], t2p[:], start=True, stop=True)
                x2 = wpool.tile([128, GH], bf16, tag="x1")
                if b2_zero:
                    nc.vector.tensor_scalar(x2[:], z2_ps[:], 0.0, dinv[:],
                                            ALU.max, ALU.mult)
                else:
                    if g == 0 and _b2b[0] is None:
                        _b2b[0] = cpool.tile([128, GH], f32, name="b2bt")
                        nc.scalar.dma_start(_b2b[0][:], b2b_d[:])
                    x2p = wpool.tile([128, GH], f32, tag="x1p")
                    nc.vector.scalar_tensor_tensor(x2p[:], z2_ps[:], dinv[:],
                                                   _b2b[0][:], ALU.mult, ALU.add)
                    nc.vector.tensor_scalar_max(x2[:], x2p[:], 0.0)

                # graph mean pool -> column g of catT6
                pool_ps = psB.tile([128, 1], f32, tag="mm")
                nc.tensor.matmul(pool_ps[:], x2[:], mean_b[:], start=True,
                                 stop=True)
                nc.scalar.copy(catT6[:, g : g + 1], pool_ps[:])

            # ---------- FC head over all BL graphs ----------
            ident_f = cpool.tile([128, 128], f32)
            nc.scalar.dma_start(ident_f[:], identf_d[:])
            wf1s = cpool.tile([128, FC, FH], f32)
            nc.scalar.dma_start(wf1s[:], wf1t_d[:])
            wf2s = cpool.tile([128, 2, L], f32)
            nc.scalar.dma_start(wf2s[:], wf2t_d[:])
            bf1b = cpool.tile([BL, FH], f32)
            nc.scalar.dma_start(bf1b[:], bf1b_d[:])
            bf2b = cpool.tile([BL, L], f32)
            nc.scalar.dma_start(bf2b[:], bf2b_d[:])
            clsr = cpool.tile([BL, H], f32)
            nc.sync.dma_start(clsr[:], lh_d[:, 0, :].bitcast(f32))
            h1_ps = psB.tile([BL, FH], f32, tag="mm")
            for c in range(FC):
                if c < HC:
                    ct_ps = psB.tile([128, BL], f32, tag="mm")
                    nc.tensor.transpose(ct_ps[:], clsr[:, c * 128 : (c + 1) * 128],
                                        ident_f[0:BL, 0:BL])
                    catc = wpool.tile([128, BL], f32, tag="catc", bufs=2)
                    nc.any.tensor_copy(catc[:], ct_ps[:])
                else:
                    catc = catT6
                nc.tensor.matmul(h1_ps[:], catc[:], wf1s[:, c, :], start=(c == 0),
                                 stop=(c == FC - 1))
            h1s = wpool.tile([BL, FH], f32, tag="h1")
            nc.vector.scalar_tensor_tensor(h1s[:], h1_ps[:], 1.0, bf1b[:],
                                           ALU.bypass, ALU.add)
            hr = wpool.tile([BL, FH], f32, tag="h1")
            nc.vector.tensor_scalar_max(hr[:], h1s[:], 0.0)
            out_ps = psB.tile([BL, L], f32, tag="mm")
            for c in range(2):
                ht_ps = psB.tile([128, BL], f32, tag="mm")
                nc.tensor.transpose(ht_ps[:], hr[:, c * 128 : (c + 1) * 128],
                                    ident_f[0:BL, 0:BL])
                htc = wpool.tile([128, BL], f32, tag="catc", bufs=2)
                nc.any.tensor_copy(htc[:], ht_ps[:])
                nc.tensor.matmul(out_ps[:], htc[:], wf2s[:, c, :], start=(c == 0),
                                 stop=(c == 1))
            outs = wpool.tile([BL, L], f32, tag="outs")
            nc.vector.scalar_tensor_tensor(outs[:], out_ps[:], 1.0, bf2b[:],
                                           ALU.bypass, ALU.add)
            nc.sync.dma_start(out_d[:], outs[:])

    _split_multi_waits(nc)
    return nc


def _prepare_in_maps(inputs):
    lh = np.ascontiguousarray(np.asarray(inputs["last_hidden"], dtype=np.float32))
    submap = np.asarray(inputs["submap"]).astype(np.int64)
    edge_index = np.asarray(inputs["edge_index"]).astype(np.int64)
    assert lh.shape == (B, S, H)
    assert int(inputs.get("num_nodes", N)) == N

    wr = np.asarray(inputs["wr"], dtype=np.float32)
    br = float(np.asarray(inputs["br"], dtype=np.float32))
    W1 = np.asarray(inputs["W1"], dtype=np.float32)
    b1 = np.asarray(inputs["b1"], dtype=np.float32)
    W2 = np.asarray(inputs["W2"], dtype=np.float32)
    b2 = np.asarray(inputs["b2"], dtype=np.float32)
    Wf1 = np.asarray(inputs["Wf1"], dtype=np.float32)
    bf1 = np.asarray(inputs["bf1"], dtype=np.float32)
    Wf2 = np.asarray(inputs["Wf2"], dtype=np.float32)
    bf2 = np.asarray(inputs["bf2"], dtype=np.float32)

    # Shared (replicated) tensors.
    consts = {
        "wrb": np.ascontiguousarray(np.broadcast_to(wr, (128, H))),
        "w1t": np.ascontiguousarray(
            W1.reshape(HC, 128, GH).transpose(1, 0, 2)).astype(BF16),
        "w2t": np.ascontiguousarray(W2).astype(BF16),
        "wf1t": np.ascontiguousarray(
            Wf1.reshape(FC, 128, FH).transpose(1, 0, 2)),
        "wf2t": np.ascontiguousarray(
            Wf2.reshape(2, 128, L).transpose(1, 0, 2)),
        "b1b": np.ascontiguousarray(np.broadcast_to(b1, (128, GH))),
        "b2b": np.ascontiguousarray(np.broadcast_to(b2, (128, GH))),
        "bf1b": np.ascontiguousarray(np.broadcast_to(bf1, (BL, FH))),
        "bf2b": np.ascontiguousarray(np.broadcast_to(bf2, (BL, L))),
        "iota_f": np.ascontiguousarray(
            np.broadcast_to(np.arange(128, dtype=np.float32), (128, 128))),
        "iota8": np.ascontiguousarray(
            np.broadcast_to(np.arange(128, dtype=np.float32), (128, EC, 128))),
        "ident_b": np.eye(128, dtype=np.float32).astype(BF16),
        "ident_f": np.eye(128, dtype=np.float32),
        "ones_r": np.ones((128, 1), np.float32),
        "ones_b": np.ones((128, 1), np.float32).astype(BF16),
        "mean_b": np.full((128, 1), 1.0 / N, np.float32).astype(BF16),
    }

    # Per-graph index layouts: value of token t goes to partition t%128,
    # column t//128.
    subv = submap.reshape(B, SC, 128).transpose(0, 2, 1).astype(np.float32)
    esrc = edge_index[:, 0, :].reshape(B, EC, 128).transpose(0, 2, 1).astype(np.float32)
    edst = edge_index[:, 1, :].reshape(B, EC, 128).transpose(0, 2, 1).astype(np.float32)

    in_maps = []
    for i in range(NCORES):
        sl = slice(i * BL, (i + 1) * BL)
        m = dict(consts)
        m["lh"] = np.ascontiguousarray(lh[sl])
        m["subv"] = np.ascontiguousarray(subv[sl])
        m["esrc"] = np.ascontiguousarray(esrc[sl])
        m["edst"] = np.ascontiguousarray(edst[sl])
        in_maps.append(m)
    flags = (br, bool(np.all(b1 == 0)), bool(np.all(b2 == 0)))
    return in_maps, flags


def _run(inputs, trace=False):
    in_maps, flags = _prepare_in_maps(inputs)
    key = ("prog",) + flags
    if key not in _CACHE:
        _CACHE[key] = build_program(*flags)
    nc = _CACHE[key]
    res = run_bass_kernel_spmd(nc, in_maps, list(range(NCORES)), trace=trace)
    out = np.concatenate([np.asarray(res.results[i]["out"]) for i in range(NCORES)],
                         axis=0).astype(np.float32)
    return out, res


def kernel(**inputs) -> np.ndarray:
    out, _ = _run(inputs, trace=False)
    return out



# revision 8
# speedup vs baseline: 1.0940x; 1.0940x over previous
"""Trainium2 Bass kernel for BioBERT-ARG-GNN (gated pooling + 2-layer GCN + MLP head).

Strategy (v2): pure data parallel over batch B=64 across 8 NeuronCores (8
graphs per core).  All index-derived structures (subtoken one-hot pooling
matrix with 1/cnt and D^-1/2 folded in, symmetric-normalized adjacency
\hat A = D^-1/2 (A+I) D^-1/2) are precomputed on the host and shipped as
bf16 alongside a bf16 copy of last_hidden in ONE mega-tensor per graph
(single DMA each).  On device, per graph: gate logits via DVE
multiply-accumulate, sigmoid on ACT, gate*P on GPSIMD, pooling + GCN +
head as bf16 matmuls (PSUM f32).  The nf [128,768] -> nfT [768,128]
transpose rides the DMA XBAR (dma transpose) instead of the PE.  The
whole per-graph chain is software-pipelined in 8+7 steps so the PE and
DVE/ACT/GPSIMD streams stay busy while the 8 graph DMAs stream in.
"""

import os
import sys

import numpy as np

for _p in ("/opt/trn_rl_repo", "/root/.axon_site/_ro/trn_rl_repo"):
    if os.path.isdir(_p) and _p not in sys.path:
        sys.path.insert(0, _p)

import ml_dtypes  # noqa: E402
import concourse.bass as bass  # noqa: E402
import concourse.mybir as mybir  # noqa: E402
from concourse import tile  # noqa: E402
from concourse.bass_utils import run_bass_kernel_spmd  # noqa: E402

# Problem shapes (hardcoded per contest rules).
B, S, H = 64, 512, 768
N, E = 128, 1024
GH, FH, L = 128, 256, 2
NCORES = 8
BL = B // NCORES  # graphs per core
SC = S // 128     # subtoken chunks per graph
HC = H // 128     # BERT-hidden chunks
FC = (H + GH) // 128  # concat-feature chunks for the FC head

# mega-tensor column offsets (bf16)
MEG_LH = 0            # [SC*H] = 3072: lh[c*768 : (c+1)*768]
MEG_PG = SC * H       # [SC*N] = 512: P' (one-hot * invc * dinv)
MEG_AH = MEG_PG + SC * N  # [N] = 128: \hat A row block
MEG_W = MEG_AH + N    # 3712 total

# consts column offsets (bf16)
C_WRB = 0             # [768] wr broadcast
C_W1 = 768            # [HC*GH] = 768
C_W2 = C_W1 + HC * GH         # [GH] = 128 cols -> 1536:1664
C_WF1 = C_W2 + GH             # [FC*2*128] = 1792 -> 1664:3456
C_WF2 = C_WF1 + FC * 2 * 128  # [2*L] = 4 -> 3456:3460
C_CLS = C_WF2 + 2 * L         # [HC*BL] = 48 -> 3460:3508
C_MEAN = C_CLS + HC * BL      # [1] -> 3508
C_IDENT = 3512               # [128] -> 3512:3640
C_W = 3640

f32 = mybir.dt.float32
bf16 = mybir.dt.bfloat16
AFT = mybir.ActivationFunctionType
ALU = mybir.AluOpType
BF16 = ml_dtypes.bfloat16

_CACHE = {}


def _split_multi_waits(nc: bass.Bass) -> int:
    """Walrus in this container accepts one sync-wait per instruction; split
    extra waits into single-wait EventSemaphore nops just before it."""
    n_split = 0
    for fn in nc.m.functions:
        for blk in fn.blocks:
            new_instrs = []
            changed = False
            for inst in blk.instructions:
                si = getattr(inst, "sync_info", None)
                if si is not None and si.on_wait is not None and len(si.on_wait) > 1:
                    waits = list(si.on_wait)
                    for j, w in enumerate(waits[:-1]):
                        ev = mybir.InstEventSemaphore(
                            name=f"{inst.name}_ws{j}",
                            ins=[], outs=[],
                            engine=inst.engine,
                            sync_info=mybir.SyncInfo(on_wait=[w], on_update=[]),
                        )
                        new_instrs.append(ev)
                    inst.sync_info = mybir.SyncInfo(
                        on_wait=[waits[-1]], on_update=list(si.on_update))
                    n_split += 1
                    changed = True
                new_instrs.append(inst)
            if changed:
                blk.instructions = new_instrs
    return n_split


def build_program(br_val: float, b1_zero: bool, b2_zero: bool,
                  bf1_zero: bool, bf2_zero: bool) -> bass.Bass:
    nc = bass.Bass()

    meg_d = nc.declare_dram_parameter("meg", [BL, 128, MEG_W], bf16, isOutput=False)
    consts_d = nc.declare_dram_parameter("consts", [128, C_W], bf16, isOutput=False)
    b1b_d = nc.declare_dram_parameter("b1b", [128, GH], f32, isOutput=False)
    b2b_d = nc.declare_dram_parameter("b2b", [128, GH], f32, isOutput=False)
    bf1b_d = nc.declare_dram_parameter("bf1b", [128, 2], f32, isOutput=False)
    bf2b_d = nc.declare_dram_parameter("bf2b", [L, 1], f32, isOutput=False)
    out_d = nc.declare_dram_parameter("out", [L, BL], f32, isOutput=True)

    MMB = 4  # psum rotation depth for small matmul outputs (bank-granular)

    with tile.TileContext(nc) as tc:
        with (
            tc.tile_pool(name="const", bufs=1) as cpool,
            tc.tile_pool(name="megp", bufs=BL) as megpool,
            tc.tile_pool(name="gate", bufs=1) as gpool,
            tc.tile_pool(name="work", bufs=3) as wpool,
            tc.tile_pool(name="psA", bufs=2, space="PSUM") as psA,
            tc.tile_pool(name="psB", bufs=MMB, space="PSUM") as psB,
        ):
            ctile = cpool.tile([128, C_W], bf16)
            nc.scalar.dma_start(ctile[:], consts_d[:])
            b1t = b2t = bf1t = bf2t = None
            if not b1_zero:
                b1t = cpool.tile([128, GH], f32, name="b1t")
                nc.scalar.dma_start(b1t[:], b1b_d[:])
            if not b2_zero:
                b2t = cpool.tile([128, GH], f32, name="b2t")
                nc.scalar.dma_start(b2t[:], b2b_d[:])
            if not bf1_zero:
                bf1t = cpool.tile([128, 2], f32, name="bf1t")
                nc.scalar.dma_start(bf1t[:], bf1b_d[:])
            if not bf2_zero:
                bf2t = cpool.tile([L, 1], f32, name="bf2t")
                nc.scalar.dma_start(bf2t[:], bf2b_d[:])
            catT6 = cpool.tile([128, BL], bf16)
            h1r = cpool.tile([128, 2, BL], bf16)

            megs = []
            for g in range(BL):
                m = megpool.tile([128, MEG_W], bf16, tag="meg", bufs=BL,
                                 name=f"meg{g}")
                nc.sync.dma_start(m[:], meg_d[g])
                megs.append(m)

            WRB = ctile[:, C_WRB:C_WRB + H]
            W2c = ctile[:, C_W2:C_W2 + GH]
            MEAN = ctile[:, C_MEAN:C_MEAN + 1]
            IDENT = ctile[:, C_IDENT:C_IDENT + 128]

            scr = gpool.tile([128, H], bf16)

            nfps = [None] * BL
            nfs = [None] * BL
            t1sb = [None] * BL
            x1 = [None] * BL
            x1t = [None] * BL
            t2sb = [None] * BL
            x2 = [None] * BL

            def relu_to(out_sb, z_ps, bias_tile, tag):
                if bias_tile is None:
                    nc.vector.tensor_scalar_max(out_sb[:], z_ps[:], 0.0)
                else:
                    tmp = wpool.tile([128, GH], f32, tag=tag + "b", bufs=2,
                                     name=tag + "b")
                    nc.vector.tensor_tensor(tmp[:], z_ps[:], bias_tile[:],
                                            ALU.add)
                    nc.vector.tensor_scalar_max(out_sb[:], tmp[:], 0.0)

            for s in range(BL + 7):
                gH, gG, gF, gE, gD, gC, gB, gA = (s - 7, s - 6, s - 5, s - 4,
                                                  s - 3, s - 2, s - 1, s)
                if 0 <= gH < BL:
                    g = gH
                    mp_ps = psB.tile([128, 1], f32, tag="mm", name="mp_ps")
                    nc.tensor.matmul(mp_ps[:], x2[g][:], MEAN,
                                     start=True, stop=True)
                    nc.vector.tensor_copy(catT6[:, g:g + 1], mp_ps[:])
                if 0 <= gG < BL:
                    g = gG
                    z2_ps = psB.tile([128, GH], f32, tag="mm", name="z2_ps")
                    nc.tensor.matmul(z2_ps[:], megs[g][:, MEG_AH:MEG_AH + N],
                                     t2sb[g][:], start=True, stop=True)
                    x2[g] = wpool.tile([128, GH], bf16, tag="x2", bufs=3,
                                       name="x2")
                    relu_to(x2[g], z2_ps, b2t, "x2")
                if 0 <= gF < BL:
                    g = gF
                    t2_ps = psB.tile([128, GH], f32, tag="mm", name="t2_ps")
                    nc.tensor.matmul(t2_ps[:], x1t[g][:], W2c,
                                     start=True, stop=True)
                    t2sb[g] = wpool.tile([128, GH], bf16, tag="t2sb", bufs=3,
                                         name="t2sb")
                    nc.vector.tensor_copy(t2sb[g][:], t2_ps[:])
                if 0 <= gE < BL:
                    g = gE
                    xt_ps = psB.tile([128, GH], bf16, tag="mm", name="xt_ps")
                    nc.tensor.transpose(xt_ps[:], x1[g][:], IDENT)
                    x1t[g] = wpool.tile([128, GH], bf16, tag="x1t", bufs=3,
                                        name="x1t")
                    nc.vector.tensor_copy(x1t[g][:], xt_ps[:])
                if 0 <= gD < BL:
                    g = gD
                    z_ps = psB.tile([128, GH], f32, tag="mm", name="z_ps")
                    nc.tensor.matmul(z_ps[:], megs[g][:, MEG_AH:MEG_AH + N],
                                     t1sb[g][:], start=True, stop=True)
                    x1[g] = wpool.tile([128, GH], bf16, tag="x1", bufs=3,
                                       name="x1")
                    relu_to(x1[g], z_ps, b1t, "x1")
                if 0 <= gC < BL:
                    g = gC
                    t1_ps = psB.tile([128, GH], f32, tag="mm", name="t1_ps")
                    for hc in range(HC):
                        nc.tensor.matmul(
                            t1_ps[:], nfs[g][:, hc, :],
                            ctile[:, C_W1 + hc * GH:C_W1 + (hc + 1) * GH],
                            start=(hc == 0), stop=(hc == HC - 1))
                    t1sb[g] = wpool.tile([128, GH], bf16, tag="t1sb", bufs=3,
                                         name="t1sb")
                    nc.scalar.copy(t1sb[g][:], t1_ps[:])
                if 0 <= gB < BL:
                    g = gB
                    nfsb = wpool.tile([128, H], bf16, tag="nfsb", bufs=2,
                                      name="nfsb")
                    nc.scalar.copy(nfsb[:], nfps[g][:])
                    nfs[g] = wpool.tile([128, HC, 128], bf16, tag="nfs",
                                        bufs=3, name="nfs")
                    nc.scalar.dma_start(nfs[g][:], nfsb[:], transpose=True)
                if 0 <= gA < BL:
                    g = gA
                    pg = wpool.tile([128, SC, 128], bf16, tag="pg", bufs=3,
                                    name="pg")
                    for c in range(SC):
                        logits = wpool.tile([128, 1], f32, tag="lg", bufs=4,
                                            name="logits")
                        nc.vector.scalar_tensor_tensor(
                            scr[:], megs[g][:, c * H:(c + 1) * H], 0.0, WRB,
                            ALU.bypass, ALU.mult, accum_out=logits[:])
                        gate = wpool.tile([128, 1], f32, tag="gt", bufs=8,
                                          name="gate")
                        nc.scalar.activation(gate[:], logits[:], AFT.Sigmoid,
                                             bias=float(br_val))
                        nc.gpsimd.tensor_scalar_mul(
                            pg[:, c, :],
                            megs[g][:, MEG_PG + c * N:MEG_PG + (c + 1) * N],
                            gate[:])
                    nfps[g] = psA.tile([128, H], f32, tag="nf", name="nf_ps")
                    for c in range(SC):
                        nc.tensor.matmul(nfps[g][:, 0:512], pg[:, c, :],
                                         megs[g][:, c * H:c * H + 512],
                                         start=(c == 0), stop=(c == SC - 1))
                        nc.tensor.matmul(nfps[g][:, 512:H], pg[:, c, :],
                                         megs[g][:, c * H + 512:(c + 1) * H],
                                         start=(c == 0), stop=(c == SC - 1))

            # ---------- FC head over all BL graphs ----------
            h1_ps = []
            for hh in range(2):
                hp = psB.tile([128, BL], f32, tag="mm", name=f"h1_ps{hh}")
                for c in range(FC):
                    lhsT = ctile[:, C_WF1 + (c * 2 + hh) * 128:
                                 C_WF1 + (c * 2 + hh + 1) * 128]
                    rhs = (ctile[:, C_CLS + c * BL:C_CLS + (c + 1) * BL]
                           if c < HC else catT6[:])
                    nc.tensor.matmul(hp[:], lhsT, rhs, start=(c == 0),
                                     stop=(c == FC - 1))
                h1_ps.append(hp)
            for hh in range(2):
                if bf1t is None:
                    nc.vector.tensor_scalar_max(h1r[:, hh, :], h1_ps[hh][:],
                                                0.0)
                else:
                    nc.vector.tensor_scalar(h1r[:, hh, :], h1_ps[hh][:],
                                            bf1t[:, hh:hh + 1], 0.0,
                                            ALU.add, ALU.max)
            out_ps = psB.tile([L, BL], f32, tag="mm", name="out_ps")
            for hh in range(2):
                nc.tensor.matmul(out_ps[:],
                                 ctile[:, C_WF2 + hh * L:C_WF2 + (hh + 1) * L],
                                 h1r[:, hh, :], start=(hh == 0),
                                 stop=(hh == 1))
            outs = cpool.tile([L, BL], f32)
            if bf2t is None:
                nc.vector.tensor_copy(outs[:], out_ps[:])
            else:
                nc.vector.tensor_scalar_add(outs[:], out_ps[:], bf2t[:])
            nc.sync.dma_start(out_d[:], outs[:])

    _split_multi_waits(nc)
    return nc


def _prepare_in_maps(inputs):
    lh = np.ascontiguousarray(np.asarray(inputs["last_hidden"], dtype=np.float32))
    submap = np.asarray(inputs["submap"]).astype(np.int64)
    edge_index = np.asarray(inputs["edge_index"]).astype(np.int64)
    assert lh.shape == (B, S, H)
    assert int(inputs.get("num_nodes", N)) == N

    wr = np.asarray(inputs["wr"], dtype=np.float32)
    br = float(np.asarray(inputs["br"], dtype=np.float32))
    W1 = np.asarray(inputs["W1"], dtype=np.float32)
    b1 = np.asarray(inputs["b1"], dtype=np.float32)
    W2 = np.asarray(inputs["W2"], dtype=np.float32)
    b2 = np.asarray(inputs["b2"], dtype=np.float32)
    Wf1 = np.asarray(inputs["Wf1"], dtype=np.float32)
    bf1 = np.asarray(inputs["bf1"], dtype=np.float32)
    Wf2 = np.asarray(inputs["Wf2"], dtype=np.float32)
    bf2 = np.asarray(inputs["bf2"], dtype=np.float32)

    # ---- host-side index prep: adjacency, degrees, counts ----
    src = edge_index[:, 0, :]
    dst = edge_index[:, 1, :]
    flat = (np.arange(B, dtype=np.int64)[:, None] * (N * N) + src * N + dst)
    A = np.bincount(flat.reshape(-1), minlength=B * N * N).astype(np.float32)
    A = A.reshape(B, N, N) + np.eye(N, dtype=np.float32)[None]
    deg = A.sum(axis=1)                      # in-degree incl self-loops
    dinv = 1.0 / np.sqrt(deg)
    ahat = A * dinv[:, :, None] * dinv[:, None, :]

    cflat = np.arange(B, dtype=np.int64)[:, None] * N + submap
    cnt = np.bincount(cflat.reshape(-1), minlength=B * N).astype(np.float32)
    invc = 1.0 / np.maximum(cnt.reshape(B, N), 1.0)

    P = (submap[:, :, None] == np.arange(N)[None, None, :]).astype(np.float32)
    P *= (invc * dinv)[:, None, :]

    # ---- mega-tensor assembly (bf16) ----
    lh_r = lh.astype(BF16).reshape(B, SC, 128, H).transpose(0, 2, 1, 3)
    p_r = P.astype(BF16).reshape(B, SC, 128, N).transpose(0, 2, 1, 3)
    meg = np.empty((B, 128, MEG_W), dtype=BF16)
    meg[:, :, MEG_LH:MEG_PG] = lh_r.reshape(B, 128, SC * H)
    meg[:, :, MEG_PG:MEG_AH] = p_r.reshape(B, 128, SC * N)
    meg[:, :, MEG_AH:MEG_W] = ahat.astype(BF16)

    # ---- consts (bf16), cls block differs per core ----
    consts = np.zeros((128, C_W), dtype=np.float32)
    consts[:, C_WRB:C_WRB + H] = wr[None, :]
    consts[:, C_W1:C_W1 + HC * GH] = (
        W1.reshape(HC, 128, GH).transpose(1, 0, 2).reshape(128, HC * GH))
    consts[:, C_W2:C_W2 + GH] = W2
    consts[:, C_WF1:C_WF1 + FC * 2 * 128] = (
        Wf1.reshape(FC, 128, 2, 128).transpose(1, 0, 2, 3).reshape(128, -1))
    consts[:, C_WF2:C_WF2 + 2 * L] = (
        Wf2.reshape(2, 128, L).transpose(1, 0, 2).reshape(128, 2 * L))
    consts[:, C_MEAN] = 1.0 / N
    consts[:, C_IDENT:C_IDENT + 128] = np.eye(128, dtype=np.float32)

    b1b = np.ascontiguousarray(np.broadcast_to(b1, (128, GH)).astype(np.float32))
    b2b = np.ascontiguousarray(np.broadcast_to(b2, (128, GH)).astype(np.float32))
    bf1b = np.ascontiguousarray(bf1.reshape(2, 128).T.astype(np.float32))
    bf2b = np.ascontiguousarray(bf2.reshape(L, 1).astype(np.float32))

    in_maps = []
    for i in range(NCORES):
        sl = slice(i * BL, (i + 1) * BL)
        ci = consts.copy()
        ci[:, C_CLS:C_CLS + HC * BL] = (
            lh[sl, 0, :].reshape(BL, HC, 128).transpose(2, 1, 0)
            .reshape(128, HC * BL))
        in_maps.append({
            "meg": np.ascontiguousarray(meg[sl]),
            "consts": ci.astype(BF16),
            "b1b": b1b, "b2b": b2b, "bf1b": bf1b, "bf2b": bf2b,
        })
    flags = (br, bool(np.all(b1 == 0)), bool(np.all(b2 == 0)),
             bool(np.all(bf1 == 0)), bool(np.all(bf2 == 0)))
    return in_maps, flags


def _run(inputs, trace=False):
    in_maps, flags = _prepare_in_maps(inputs)
    key = ("prog",) + flags
    if key not in _CACHE:
        _CACHE[key] = build_program(*flags)
    nc = _CACHE[key]
    res = run_bass_kernel_spmd(nc, in_maps, list(range(NCORES)), trace=trace)
    out = np.concatenate(
        [np.asarray(res.results[i]["out"]).T for i in range(NCORES)],
        axis=0).astype(np.float32)
    return out, res


def kernel(**inputs) -> np.ndarray:
    out, _ = _run(inputs, trace=False)
    return out


# revision 9
# speedup vs baseline: 1.7612x; 1.6099x over previous
"""Trainium2 Bass kernel for BioBERT-ARG-GNN (gated pooling + 2-layer GCN + MLP head).

Strategy (v3): pure data parallel over batch B=64 across 8 NeuronCores (8
graphs per core).  Host precomputes index-derived structures (one-hot
pooling matrix P' with 1/cnt and D^-1/2 folded in, normalized adjacency
\hat A = D^-1/2 (A+I) D^-1/2) and ships them bf16 together with a bf16
TRANSPOSED copy of last_hidden (lhT, [hidden, tokens]) in ONE mega-tensor
per graph.  The transposed layout lets BOTH the gate logits (wr . lh_t)
and the W1 projection run on the PE as matmuls with the contraction over
the hidden dim:

    yT[gh, t]  = sum_hc W1c^T @ lhT_c          (6 matmuls, free=512)
    lg[0:1, t] = sum_hc wr_c^T @ lhT_c         (6 matmuls, free=512, 16-wide)

Both results ride the DMA XBAR back to token-major layout; sigmoid(lg)
gives per-token gates, gates fold into P' (DVE tensor_scalar, 2x bf16),
and pooling contracts tokens directly into t1 = P'^T (g*y) = nf @ W1 —
skipping the nf materialization + transpose of the classic order.  The
GCN layers use \hat A as the stationary operand; the FC head is batched
over all 8 graphs with zero device-side transposes.  Everything is
software-pipelined in 9 stages across the 8 graph DMAs (SP + GPSIMD
SWDGE rings alternate to keep the 16 DMA engines fed).
"""

import os
import sys

import numpy as np

for _p in ("/opt/trn_rl_repo", "/root/.axon_site/_ro/trn_rl_repo"):
    if os.path.isdir(_p) and _p not in sys.path:
        sys.path.insert(0, _p)

import ml_dtypes  # noqa: E402
import concourse.bass as bass  # noqa: E402
import concourse.mybir as mybir  # noqa: E402
from concourse import tile  # noqa: E402
from concourse.bass_utils import run_bass_kernel_spmd  # noqa: E402

# Problem shapes (hardcoded per contest rules).
B, S, H = 64, 512, 768
N, E = 128, 1024
GH, FH, L = 128, 256, 2
NCORES = 8
BL = B // NCORES  # graphs per core
SC = S // 128     # subtoken chunks per graph
HC = H // 128     # BERT-hidden chunks
FC = (H + GH) // 128  # concat-feature chunks for the FC head

# mega-tensor column offsets (bf16)
MEG_LHT = 0             # [HC*S] = 3072: lhT[p, hc*S + t] = lh[t, hc*128+p]
MEG_PG = HC * S         # [SC*N] = 512: P' (one-hot * invc * dinv), token-major
MEG_AH = MEG_PG + SC * N  # [N] = 128: \hat A row block
MEG_W = MEG_AH + N      # 3712 total

# consts column offsets (bf16)
C_W1 = 0                      # [HC*GH] = 768: [p, hc*128+j] = W1[hc*128+p, j]
C_W2 = C_W1 + HC * GH         # [GH]
C_WF1 = C_W2 + GH             # [FC*2*128] = 1792
C_WF2 = C_WF1 + FC * 2 * 128  # [2*L] = 4
C_CLS = C_WF2 + 2 * L         # [HC*BL] = 48
C_MEAN = C_CLS + HC * BL      # [1]
C_WR = C_MEAN + 12            # [HC*16] = 96: [p, hc*16+0] = wr[hc*128+p]
C_IDENT = C_WR + HC * 16      # [128]
C_W = C_IDENT + 128

f32 = mybir.dt.float32
bf16 = mybir.dt.bfloat16
AFT = mybir.ActivationFunctionType
ALU = mybir.AluOpType
BF16 = ml_dtypes.bfloat16

_CACHE = {}


def _split_multi_waits(nc: bass.Bass) -> int:
    """Walrus in this container accepts one sync-wait per instruction; split
    extra waits into single-wait EventSemaphore nops just before it."""
    n_split = 0
    for fn in nc.m.functions:
        for blk in fn.blocks:
            new_instrs = []
            changed = False
            for inst in blk.instructions:
                si = getattr(inst, "sync_info", None)
                if si is not None and si.on_wait is not None and len(si.on_wait) > 1:
                    waits = list(si.on_wait)
                    for j, w in enumerate(waits[:-1]):
                        ev = mybir.InstEventSemaphore(
                            name=f"{inst.name}_ws{j}",
                            ins=[], outs=[],
                            engine=inst.engine,
                            sync_info=mybir.SyncInfo(on_wait=[w], on_update=[]),
                        )
                        new_instrs.append(ev)
                    inst.sync_info = mybir.SyncInfo(
                        on_wait=[waits[-1]], on_update=list(si.on_update))
                    n_split += 1
                    changed = True
                new_instrs.append(inst)
            if changed:
                blk.instructions = new_instrs
    return n_split


def build_program(br_val: float, b1_zero: bool, b2_zero: bool,
                  bf1_zero: bool, bf2_zero: bool) -> bass.Bass:
    nc = bass.Bass()

    meg_d = nc.declare_dram_parameter("meg", [BL, 128, MEG_W], bf16, isOutput=False)
    consts_d = nc.declare_dram_parameter("consts", [128, C_W], bf16, isOutput=False)
    b1b_d = nc.declare_dram_parameter("b1b", [128, GH], f32, isOutput=False)
    b2b_d = nc.declare_dram_parameter("b2b", [128, GH], f32, isOutput=False)
    bf1b_d = nc.declare_dram_parameter("bf1b", [128, 2], f32, isOutput=False)
    bf2b_d = nc.declare_dram_parameter("bf2b", [L, 1], f32, isOutput=False)
    out_d = nc.declare_dram_parameter("out", [L, BL], f32, isOutput=True)

    with tile.TileContext(nc) as tc:
        with (
            tc.tile_pool(name="const", bufs=1) as cpool,
            tc.tile_pool(name="megp", bufs=BL) as megpool,
            tc.tile_pool(name="work", bufs=3) as wpool,
            tc.tile_pool(name="psY", bufs=2, space="PSUM") as psY,
            tc.tile_pool(name="psL", bufs=2, space="PSUM") as psL,
            tc.tile_pool(name="psB", bufs=4, space="PSUM") as psB,
        ):
            ctile = cpool.tile([128, C_W], bf16)
            nc.scalar.dma_start(ctile[:], consts_d[:])
            b1t = b2t = bf1t = bf2t = None
            if not b1_zero:
                b1t = cpool.tile([128, GH], f32, name="b1t")
                nc.scalar.dma_start(b1t[:], b1b_d[:])
            if not b2_zero:
                b2t = cpool.tile([128, GH], f32, name="b2t")
                nc.scalar.dma_start(b2t[:], b2b_d[:])
            if not bf1_zero:
                bf1t = cpool.tile([128, 2], f32, name="bf1t")
                nc.scalar.dma_start(bf1t[:], bf1b_d[:])
            if not bf2_zero:
                bf2t = cpool.tile([L, 1], f32, name="bf2t")
                nc.scalar.dma_start(bf2t[:], bf2b_d[:])
            catT6 = cpool.tile([128, BL], bf16)
            h1r = cpool.tile([128, 2, BL], bf16)

            megs = []
            for g in range(BL):
                m = megpool.tile([128, MEG_W], bf16, tag="meg", bufs=BL,
                                 name=f"meg{g}")
                if g % 2 == 0:
                    nc.sync.dma_start(m[:], meg_d[g])
                else:
                    nc.gpsimd.dma_start(m[:], meg_d[g])
                megs.append(m)

            W2c = ctile[:, C_W2:C_W2 + GH]
            MEAN = ctile[:, C_MEAN:C_MEAN + 1]
            IDENT = ctile[:, C_IDENT:C_IDENT + 128]

            yT_ps = [None] * BL
            lg_ps = [None] * BL
            y_sb = [None] * BL
            gT = [None] * BL
            gates = [None] * BL
            t1sb = [None] * BL
            x1 = [None] * BL
            x1t = [None] * BL
            t2sb = [None] * BL
            x2 = [None] * BL

            def relu_to(out_sb, z_ps, bias_tile, tag):
                if bias_tile is None:
                    nc.vector.tensor_scalar_max(out_sb[:], z_ps[:], 0.0)
                else:
                    tmp = wpool.tile([128, GH], f32, tag=tag + "b", bufs=2,
                                     name=tag + "b")
                    nc.vector.tensor_tensor(tmp[:], z_ps[:], bias_tile[:],
                                            ALU.add)
                    nc.vector.tensor_scalar_max(out_sb[:], tmp[:], 0.0)

            NST = 9  # pipeline stages
            for s in range(BL + NST - 1):
                gI, gH_, gG, gF, gE, gD, gC, gB, gA = (
                    s - 8, s - 7, s - 6, s - 5, s - 4, s - 3, s - 2, s - 1, s)
                if 0 <= gI < BL:     # stage I: mean-pool column
                    g = gI
                    mp_ps = psB.tile([128, 1], f32, tag="mm", name="mp_ps")
                    nc.tensor.matmul(mp_ps[:], x2[g][:], MEAN,
                                     start=True, stop=True)
                    nc.vector.tensor_copy(catT6[:, g:g + 1], mp_ps[:])
                if 0 <= gH_ < BL:    # stage H: GCN layer-2 aggregate + relu
                    g = gH_
                    z2_ps = psB.tile([128, GH], f32, tag="mm", name="z2_ps")
                    nc.tensor.matmul(z2_ps[:], megs[g][:, MEG_AH:MEG_AH + N],
                                     t2sb[g][:], start=True, stop=True)
                    x2[g] = wpool.tile([128, GH], bf16, tag="x2", bufs=3,
                                       name="x2")
                    relu_to(x2[g], z2_ps, b2t, "x2")
                if 0 <= gG < BL:     # stage G: x1 @ W2
                    g = gG
                    t2_ps = psB.tile([128, GH], f32, tag="mm", name="t2_ps")
                    nc.tensor.matmul(t2_ps[:], x1t[g][:], W2c,
                                     start=True, stop=True)
                    t2sb[g] = wpool.tile([128, GH], bf16, tag="t2sb", bufs=3,
                                         name="t2sb")
                    nc.scalar.copy(t2sb[g][:], t2_ps[:])
                if 0 <= gF < BL:     # stage F: transpose x1
                    g = gF
                    xt_ps = psB.tile([128, GH], bf16, tag="mm", name="xt_ps")
                    nc.tensor.transpose(xt_ps[:], x1[g][:], IDENT)
                    x1t[g] = wpool.tile([128, GH], bf16, tag="x1t", bufs=3,
                                        name="x1t")
                    nc.vector.tensor_copy(x1t[g][:], xt_ps[:])
                if 0 <= gE < BL:     # stage E: GCN layer-1 aggregate + relu
                    g = gE
                    z_ps = psB.tile([128, GH], f32, tag="mm", name="z_ps")
                    nc.tensor.matmul(z_ps[:], megs[g][:, MEG_AH:MEG_AH + N],
                                     t1sb[g][:], start=True, stop=True)
                    x1[g] = wpool.tile([128, GH], bf16, tag="x1", bufs=3,
                                       name="x1")
                    relu_to(x1[g], z_ps, b1t, "x1")
                if 0 <= gD < BL:     # stage D: gated pooling (t1 = P'^T g y)
                    g = gD
                    pg = wpool.tile([128, SC, 128], bf16, tag="pg", bufs=3,
                                    name="pg")
                    for c in range(SC):
                        nc.vector.tensor_scalar_mul(
                            pg[:, c, :],
                            megs[g][:, MEG_PG + c * N:MEG_PG + (c + 1) * N],
                            gates[g][:, c:c + 1])
                    t1_ps = psB.tile([128, GH], f32, tag="mm", name="t1_ps")
                    for c in range(SC):
                        nc.tensor.matmul(t1_ps[:], pg[:, c, :],
                                         y_sb[g][:, c, :],
                                         start=(c == 0), stop=(c == SC - 1))
                    t1sb[g] = wpool.tile([128, GH], bf16, tag="t1sb", bufs=3,
                                         name="t1sb")
                    nc.scalar.copy(t1sb[g][:], t1_ps[:])
                if 0 <= gC < BL:     # stage C: sigmoid gates
                    g = gC
                    gates[g] = wpool.tile([128, SC], f32, tag="gt", bufs=3,
                                          name="gates")
                    nc.scalar.activation(gates[g][:], gT[g][:, :, 0],
                                         AFT.Sigmoid, bias=float(br_val))
                if 0 <= gB < BL:     # stage B: PSUM->SBUF + XBAR transposes
                    g = gB
                    yT_sb = wpool.tile([128, S], bf16, tag="ytsb", bufs=2,
                                       name="yT_sb")
                    nc.vector.tensor_copy(yT_sb[:], yT_ps[g][:])
                    lg_sb = wpool.tile([16, S], bf16, tag="lgsb", bufs=2,
                                       name="lg_sb")
                    nc.vector.tensor_copy(lg_sb[:], lg_ps[g][:])
                    y_sb[g] = wpool.tile([128, SC, 128], bf16, tag="ysb",
                                         bufs=3, name="y_sb")
                    nc.scalar.dma_start(y_sb[g][:], yT_sb[:], transpose=True)
                    gT[g] = wpool.tile([128, SC, 16], bf16, tag="gTt", bufs=3,
                                       name="gT")
                    nc.scalar.dma_start(gT[g][:], lg_sb[:], transpose=True)
                if 0 <= gA < BL:     # stage A: W1 projection + gate logits
                    g = gA
                    yT_ps[g] = psY.tile([128, S], f32, tag="yt", name="yT_ps")
                    lg_ps[g] = psL.tile([16, S], f32, tag="lg", name="lg_ps")
                    for hc in range(HC):
                        lht_c = megs[g][:, MEG_LHT + hc * S:MEG_LHT + (hc + 1) * S]
                        nc.tensor.matmul(
                            yT_ps[g][:],
                            ctile[:, C_W1 + hc * GH:C_W1 + (hc + 1) * GH],
                            lht_c, start=(hc == 0), stop=(hc == HC - 1))
                        nc.tensor.matmul(
                            lg_ps[g][:],
                            ctile[:, C_WR + hc * 16:C_WR + (hc + 1) * 16],
                            lht_c, start=(hc == 0), stop=(hc == HC - 1))

            # ---------- FC head over all BL graphs ----------
            h1_ps = []
            for hh in range(2):
                hp = psB.tile([128, BL], f32, tag="mm", name=f"h1_ps{hh}")
                for c in range(FC):
                    lhsT = ctile[:, C_WF1 + (c * 2 + hh) * 128:
                                 C_WF1 + (c * 2 + hh + 1) * 128]
                    rhs = (ctile[:, C_CLS + c * BL:C_CLS + (c + 1) * BL]
                           if c < HC else catT6[:])
                    nc.tensor.matmul(hp[:], lhsT, rhs, start=(c == 0),
                                     stop=(c == FC - 1))
                h1_ps.append(hp)
            for hh in range(2):
                if bf1t is None:
                    nc.vector.tensor_scalar_max(h1r[:, hh, :], h1_ps[hh][:],
                                                0.0)
                else:
                    nc.vector.tensor_scalar(h1r[:, hh, :], h1_ps[hh][:],
                                            bf1t[:, hh:hh + 1], 0.0,
                                            ALU.add, ALU.max)
            out_ps = psB.tile([L, BL], f32, tag="mm", name="out_ps")
            for hh in range(2):
                nc.tensor.matmul(out_ps[:],
                                 ctile[:, C_WF2 + hh * L:C_WF2 + (hh + 1) * L],
                                 h1r[:, hh, :], start=(hh == 0),
                                 stop=(hh == 1))
            outs = cpool.tile([L, BL], f32)
            if bf2t is None:
                nc.vector.tensor_copy(outs[:], out_ps[:])
            else:
                nc.vector.tensor_scalar_add(outs[:], out_ps[:], bf2t[:])
            nc.sync.dma_start(out_d[:], outs[:])

    _split_multi_waits(nc)
    return nc


def _prepare_in_maps(inputs):
    lh = np.ascontiguousarray(np.asarray(inputs["last_hidden"], dtype=np.float32))
    submap = np.asarray(inputs["submap"]).astype(np.int64)
    edge_index = np.asarray(inputs["edge_index"]).astype(np.int64)
    assert lh.shape == (B, S, H)
    assert int(inputs.get("num_nodes", N)) == N

    wr = np.asarray(inputs["wr"], dtype=np.float32)
    br = float(np.asarray(inputs["br"], dtype=np.float32))
    W1 = np.asarray(inputs["W1"], dtype=np.float32)
    b1 = np.asarray(inputs["b1"], dtype=np.float32)
    W2 = np.asarray(inputs["W2"], dtype=np.float32)
    b2 = np.asarray(inputs["b2"], dtype=np.float32)
    Wf1 = np.asarray(inputs["Wf1"], dtype=np.float32)
    bf1 = np.asarray(inputs["bf1"], dtype=np.float32)
    Wf2 = np.asarray(inputs["Wf2"], dtype=np.float32)
    bf2 = np.asarray(inputs["bf2"], dtype=np.float32)

    # ---- host-side index prep: adjacency, degrees, counts ----
    src = edge_index[:, 0, :]
    dst = edge_index[:, 1, :]
    flat = (np.arange(B, dtype=np.int64)[:, None] * (N * N) + src * N + dst)
    A = np.bincount(flat.reshape(-1), minlength=B * N * N).astype(np.float32)
    A = A.reshape(B, N, N) + np.eye(N, dtype=np.float32)[None]
    deg = A.sum(axis=1)                      # in-degree incl self-loops
    dinv = 1.0 / np.sqrt(deg)
    ahat = A * dinv[:, :, None] * dinv[:, None, :]

    cflat = np.arange(B, dtype=np.int64)[:, None] * N + submap
    cnt = np.bincount(cflat.reshape(-1), minlength=B * N).astype(np.float32)
    invc = 1.0 / np.maximum(cnt.reshape(B, N), 1.0)

    P = (submap[:, :, None] == np.arange(N)[None, None, :]).astype(np.float32)
    P *= (invc * dinv)[:, None, :]

    # ---- mega-tensor assembly (bf16) ----
    lht = lh.astype(BF16).reshape(B, S, HC, 128).transpose(0, 3, 2, 1)
    p_r = P.astype(BF16).reshape(B, SC, 128, N).transpose(0, 2, 1, 3)
    meg = np.empty((B, 128, MEG_W), dtype=BF16)
    meg[:, :, MEG_LHT:MEG_PG] = lht.reshape(B, 128, HC * S)
    meg[:, :, MEG_PG:MEG_AH] = p_r.reshape(B, 128, SC * N)
    meg[:, :, MEG_AH:MEG_W] = ahat.astype(BF16)

    # ---- consts (bf16), cls block differs per core ----
    consts = np.zeros((128, C_W), dtype=np.float32)
    consts[:, C_W1:C_W1 + HC * GH] = (
        W1.reshape(HC, 128, GH).transpose(1, 0, 2).reshape(128, HC * GH))
    consts[:, C_W2:C_W2 + GH] = W2
    consts[:, C_WF1:C_WF1 + FC * 2 * 128] = (
        Wf1.reshape(FC, 128, 2, 128).transpose(1, 0, 2, 3).reshape(128, -1))
    consts[:, C_WF2:C_WF2 + 2 * L] = (
        Wf2.reshape(2, 128, L).transpose(1, 0, 2).reshape(128, 2 * L))
    consts[:, C_MEAN] = 1.0 / N
    wr_r = wr.reshape(HC, 128).T              # [128, HC]
    consts[:, C_WR:C_WR + HC * 16:16] = wr_r
    consts[:, C_IDENT:C_IDENT + 128] = np.eye(128, dtype=np.float32)

    b1b = np.ascontiguousarray(np.broadcast_to(b1, (128, GH)).astype(np.float32))
    b2b = np.ascontiguousarray(np.broadcast_to(b2, (128, GH)).astype(np.float32))
    bf1b = np.ascontiguousarray(bf1.reshape(2, 128).T.astype(np.float32))
    bf2b = np.ascontiguousarray(bf2.reshape(L, 1).astype(np.float32))

    in_maps = []
    for i in range(NCORES):
        sl = slice(i * BL, (i + 1) * BL)
        ci = consts.copy()
        ci[:, C_CLS:C_CLS + HC * BL] = (
            lh[sl, 0, :].reshape(BL, HC, 128).transpose(2, 1, 0)
            .reshape(128, HC * BL))
        in_maps.append({
            "meg": np.ascontiguousarray(meg[sl]),
            "consts": ci.astype(BF16),
            "b1b": b1b, "b2b": b2b, "bf1b": bf1b, "bf2b": bf2b,
        })
    flags = (br, bool(np.all(b1 == 0)), bool(np.all(b2 == 0)),
             bool(np.all(bf1 == 0)), bool(np.all(bf2 == 0)))
    return in_maps, flags


def _run(inputs, trace=False):
    in_maps, flags = _prepare_in_maps(inputs)
    key = ("prog",) + flags
    if key not in _CACHE:
        _CACHE[key] = build_program(*flags)
    nc = _CACHE[key]
    res = run_bass_kernel_spmd(nc, in_maps, list(range(NCORES)), trace=trace)
    out = np.concatenate(
        [np.asarray(res.results[i]["out"]).T for i in range(NCORES)],
        axis=0).astype(np.float32)
    return out, res


def kernel(**inputs) -> np.ndarray:
    out, _ = _run(inputs, trace=False)
    return out


# revision 12
# speedup vs baseline: 1.9737x; 1.1206x over previous
"""Trainium2 Bass kernel for BioBERT-ARG-GNN (gated pooling + 2-layer GCN + MLP head).

Strategy (v4): pure data parallel over batch B=64 across 8 NeuronCores (8
graphs per core).  Host precomputes index-derived structures (one-hot
pooling matrix P' with 1/cnt and D^-1/2 folded in, normalized adjacency
\hat A = D^-1/2 (A+I) D^-1/2) and ships them bf16 together with a bf16
TRANSPOSED copy of last_hidden (lhT, [hidden, tokens]) in ONE mega-tensor
per graph.  The transposed layout lets BOTH the gate logits (wr . lh_t)
and the W1 projection run on the PE with the contraction over hidden:

    yT[gh, t]  = sum_hc W1c^T @ lhT_c          (6 matmuls, free=512)
    lg[0:1, t] = sum_hc wr_c^T @ lhT_c         (6 matmuls, free=512)

sigmoid(lg) -> gate row [1, 512]; a 1-row matmul against a ones vector
broadcasts it to [128, 512]; DVE multiplies it into yT while casting to
bf16 (ygT); ONE DMA-XBAR transpose per graph turns ygT into token-major
y chunks; pooling then contracts tokens directly: t1 = P'^T (g*y) =
(pool(gated lh)) @ W1 — no nf materialization, no PE transposes.  GCN
layers use \hat A as stationary; FC head is batched over all 8 graphs
with no transposes.  Phase A (projection matmuls) streams back-to-back
paced by the 8 graph DMAs (SP + GPSIMD SWDGE rings alternate), keeping
the PE continuously busy; phase B runs pooling + GCN in per-stage rounds
across graphs so every PE op's cross-engine inputs are a full round old.
"""

import os
import sys

import numpy as np

for _p in ("/opt/trn_rl_repo", "/root/.axon_site/_ro/trn_rl_repo"):
    if os.path.isdir(_p) and _p not in sys.path:
        sys.path.insert(0, _p)

import ml_dtypes  # noqa: E402
import concourse.bass as bass  # noqa: E402
import concourse.mybir as mybir  # noqa: E402
from concourse import tile  # noqa: E402
from concourse.bass_utils import run_bass_kernel_spmd  # noqa: E402

# Problem shapes (hardcoded per contest rules).
B, S, H = 64, 512, 768
N, E = 128, 1024
GH, FH, L = 128, 256, 2
NCORES = 8
BL = B // NCORES  # graphs per core
SC = S // 128     # subtoken chunks per graph
HC = H // 128     # BERT-hidden chunks
FC = (H + GH) // 128  # concat-feature chunks for the FC head

# mega-tensor column offsets (bf16)
MEG_LHT = 0             # [HC*S] = 3072: lhT[p, hc*S + t] = lh[t, hc*128+p]
MEG_PG = HC * S         # [SC*N] = 512: P' (one-hot * invc * dinv), token-major
MEG_AH = MEG_PG + SC * N  # [N] = 128: \hat A row block
MEG_W = MEG_AH + N      # 3712 total

# consts column offsets (bf16)
C_W1 = 0                      # [HC*GH] = 768: [p, hc*128+j] = W1[hc*128+p, j]
C_W2 = C_W1 + HC * GH         # [GH]
C_WF1 = C_W2 + GH             # [FC*2*128] = 1792
C_WF2 = C_WF1 + FC * 2 * 128  # [2*L] = 4
C_CLS = C_WF2 + 2 * L         # [HC*BL] = 48
C_MEAN = C_CLS + HC * BL      # [1]
C_WR = C_MEAN + 12            # [HC] = 6: [p, hc] = wr[hc*128+p]
C_ONES = C_WR + HC + 10       # [128] of 1.0 (row 0 used as [1,128] lhsT)
C_IDENT = C_ONES + 128        # [128]
C_W = C_IDENT + 128

f32 = mybir.dt.float32
bf16 = mybir.dt.bfloat16
AFT = mybir.ActivationFunctionType
ALU = mybir.AluOpType
BF16 = ml_dtypes.bfloat16

_CACHE = {}


def _split_multi_waits(nc: bass.Bass) -> int:
    """Walrus in this container accepts one sync-wait per instruction; split
    extra waits into single-wait EventSemaphore nops just before it."""
    n_split = 0
    for fn in nc.m.functions:
        for blk in fn.blocks:
            new_instrs = []
            changed = False
            for inst in blk.instructions:
                si = getattr(inst, "sync_info", None)
                if si is not None and si.on_wait is not None and len(si.on_wait) > 1:
                    waits = list(si.on_wait)
                    for j, w in enumerate(waits[:-1]):
                        ev = mybir.InstEventSemaphore(
                            name=f"{inst.name}_ws{j}",
                            ins=[], outs=[],
                            engine=inst.engine,
                            sync_info=mybir.SyncInfo(on_wait=[w], on_update=[]),
                        )
                        new_instrs.append(ev)
                    inst.sync_info = mybir.SyncInfo(
                        on_wait=[waits[-1]], on_update=list(si.on_update))
                    n_split += 1
                    changed = True
                new_instrs.append(inst)
            if changed:
                blk.instructions = new_instrs
    return n_split


def build_program(br_val: float, b1_zero: bool, b2_zero: bool,
                  bf1_zero: bool, bf2_zero: bool) -> bass.Bass:
    nc = bass.Bass()

    meg_d = nc.declare_dram_parameter("meg", [BL, 128, MEG_W], bf16, isOutput=False)
    consts_d = nc.declare_dram_parameter("consts", [128, C_W], bf16, isOutput=False)
    b1b_d = nc.declare_dram_parameter("b1b", [128, GH], f32, isOutput=False)
    b2b_d = nc.declare_dram_parameter("b2b", [128, GH], f32, isOutput=False)
    bf1b_d = nc.declare_dram_parameter("bf1b", [128, 2], f32, isOutput=False)
    bf2b_d = nc.declare_dram_parameter("bf2b", [L, 1], f32, isOutput=False)
    out_d = nc.declare_dram_parameter("out", [L, BL], f32, isOutput=True)

    with tile.TileContext(nc) as tc:
        with (
            tc.tile_pool(name="const", bufs=1) as cpool,
            tc.tile_pool(name="megp", bufs=BL) as megpool,
            tc.tile_pool(name="work", bufs=3) as wpool,
            tc.tile_pool(name="psY", bufs=2, space="PSUM") as psY,
            tc.tile_pool(name="psG", bufs=2, space="PSUM") as psG,
            tc.tile_pool(name="psL", bufs=1, space="PSUM") as psL,
            tc.tile_pool(name="psB", bufs=3, space="PSUM") as psB,
        ):
            ctile = cpool.tile([128, C_W], bf16)
            nc.scalar.dma_start(ctile[:], consts_d[:])
            b1t = b2t = bf1t = bf2t = None
            if not b1_zero:
                b1t = cpool.tile([128, GH], f32, name="b1t")
                nc.scalar.dma_start(b1t[:], b1b_d[:])
            if not b2_zero:
                b2t = cpool.tile([128, GH], f32, name="b2t")
                nc.scalar.dma_start(b2t[:], b2b_d[:])
            if not bf1_zero:
                bf1t = cpool.tile([128, 2], f32, name="bf1t")
                nc.scalar.dma_start(bf1t[:], bf1b_d[:])
            if not bf2_zero:
                bf2t = cpool.tile([L, 1], f32, name="bf2t")
                nc.scalar.dma_start(bf2t[:], bf2b_d[:])
            catT6 = cpool.tile([128, BL], bf16)
            h1r = cpool.tile([128, 2, BL], bf16)

            megs = []
            for g in range(BL):
                m = megpool.tile([128, MEG_W], bf16, tag="meg", bufs=BL,
                                 name=f"meg{g}")
                if g % 2 == 0:
                    nc.sync.dma_start(m[:], meg_d[g])
                else:
                    nc.gpsimd.dma_start(m[:], meg_d[g])
                megs.append(m)

            W2c = ctile[:, C_W2:C_W2 + GH]
            MEAN = ctile[:, C_MEAN:C_MEAN + 1]
            IDENT = ctile[:, C_IDENT:C_IDENT + 128]
            ONES1 = ctile[0:1, C_ONES:C_ONES + 128]

            yT_ps = [None] * BL
            lg_ps = [None] * BL
            gate_sb = [None] * BL
            y_sb = [None] * BL
            t1sb = [None] * BL
            x1 = [None] * BL
            x1t = [None] * BL
            t2sb = [None] * BL
            x2 = [None] * BL

            def relu_to(out_sb, z_ps, bias_tile, tag):
                if bias_tile is None:
                    nc.vector.tensor_scalar_max(out_sb[:], z_ps[:], 0.0)
                else:
                    tmp = wpool.tile([128, GH], f32, tag=tag + "b", bufs=2,
                                     name=tag + "b")
                    nc.vector.tensor_tensor(tmp[:], z_ps[:], bias_tile[:],
                                            ALU.add)
                    nc.vector.tensor_scalar_max(out_sb[:], tmp[:], 0.0)

            def stage_bc(g):
                """sigmoid -> broadcast -> gate into yT -> XBAR transpose."""
                gate_sb[g] = wpool.tile([1, S], bf16, tag="gt", bufs=2,
                                        name="gate_sb")
                nc.scalar.activation(gate_sb[g][:], lg_ps[g][:], AFT.Sigmoid,
                                     bias=float(br_val))
                gb_ps = psG.tile([128, S], f32, tag="gb", name="gb_ps")
                nc.tensor.matmul(gb_ps[:], ONES1, gate_sb[g][:],
                                 start=True, stop=True)
                gb_sb = wpool.tile([128, S], bf16, tag="gbsb", bufs=2,
                                   name="gb_sb")
                nc.vector.tensor_copy(gb_sb[:], gb_ps[:])
                ygsb = wpool.tile([128, S], bf16, tag="ygsb", bufs=2,
                                  name="ygsb")
                nc.vector.tensor_tensor(ygsb[:], yT_ps[g][:], gb_sb[:],
                                        ALU.mult)
                y_sb[g] = wpool.tile([128, SC, 128], bf16, tag="ysb",
                                     bufs=BL, name="y_sb")
                nc.scalar.dma_start(y_sb[g][:], ygsb[:], transpose=True)

            # ---- phase A: projection + gate, paced by the meg DMAs ----
            for s in range(BL):
                yT_ps[s] = psY.tile([128, S], f32, tag="yt", name="yT_ps")
                lg_ps[s] = psL.tile([1, S], f32, tag="lg", name="lg_ps")
                for hc in range(HC):
                    lht_c = megs[s][:, MEG_LHT + hc * S:MEG_LHT + (hc + 1) * S]
                    nc.tensor.matmul(
                        yT_ps[s][:],
                        ctile[:, C_W1 + hc * GH:C_W1 + (hc + 1) * GH],
                        lht_c, start=(hc == 0), stop=(hc == HC - 1))
                    nc.tensor.matmul(
                        lg_ps[s][:], ctile[:, C_WR + hc:C_WR + hc + 1],
                        lht_c, start=(hc == 0), stop=(hc == HC - 1))
                if s >= 1:
                    stage_bc(s - 1)
            stage_bc(BL - 1)

            # ---- phase B: pooling + GCN in rounds across graphs ----
            for g in range(BL):
                t1_ps = psB.tile([128, GH], f32, tag="mm", name="t1_ps")
                for c in range(SC):
                    nc.tensor.matmul(t1_ps[:],
                                     megs[g][:, MEG_PG + c * N:MEG_PG + (c + 1) * N],
                                     y_sb[g][:, c, :],
                                     start=(c == 0), stop=(c == SC - 1))
                t1sb[g] = wpool.tile([128, GH], bf16, tag="t1sb", bufs=BL,
                                     name="t1sb")
                nc.scalar.copy(t1sb[g][:], t1_ps[:])
            for g in range(BL):
                z_ps = psB.tile([128, GH], f32, tag="mm", name="z_ps")
                nc.tensor.matmul(z_ps[:], megs[g][:, MEG_AH:MEG_AH + N],
                                 t1sb[g][:], start=True, stop=True)
                x1[g] = wpool.tile([128, GH], bf16, tag="x1", bufs=BL,
                                   name="x1")
                relu_to(x1[g], z_ps, b1t, "x1")
            for g in range(BL):
                xt_ps = psB.tile([128, GH], bf16, tag="mm", name="xt_ps")
                nc.tensor.transpose(xt_ps[:], x1[g][:], IDENT)
                x1t[g] = wpool.tile([128, GH], bf16, tag="x1t", bufs=BL,
                                    name="x1t")
                nc.vector.tensor_copy(x1t[g][:], xt_ps[:])
            for g in range(BL):
                t2_ps = psB.tile([128, GH], f32, tag="mm", name="t2_ps")
                nc.tensor.matmul(t2_ps[:], x1t[g][:], W2c,
                                 start=True, stop=True)
                t2sb[g] = wpool.tile([128, GH], bf16, tag="t2sb", bufs=BL,
                                     name="t2sb")
                nc.scalar.copy(t2sb[g][:], t2_ps[:])
            for g in range(BL):
                z2_ps = psB.tile([128, GH], f32, tag="mm", name="z2_ps")
                nc.tensor.matmul(z2_ps[:], megs[g][:, MEG_AH:MEG_AH + N],
                                 t2sb[g][:], start=True, stop=True)
                x2[g] = wpool.tile([128, GH], bf16, tag="x2", bufs=BL,
                                   name="x2")
                relu_to(x2[g], z2_ps, b2t, "x2")
            for g in range(BL):
                mp_ps = psB.tile([128, 1], f32, tag="mm", name="mp_ps")
                nc.tensor.matmul(mp_ps[:], x2[g][:], MEAN,
                                 start=True, stop=True)
                nc.vector.tensor_copy(catT6[:, g:g + 1], mp_ps[:])

            # ---------- FC head over all BL graphs ----------
            h1_ps = []
            for hh in range(2):
                hp = psB.tile([128, BL], f32, tag="mm", name=f"h1_ps{hh}")
                for c in range(FC):
                    lhsT = ctile[:, C_WF1 + (c * 2 + hh) * 128:
                                 C_WF1 + (c * 2 + hh + 1) * 128]
                    rhs = (ctile[:, C_CLS + c * BL:C_CLS + (c + 1) * BL]
                           if c < HC else catT6[:])
                    nc.tensor.matmul(hp[:], lhsT, rhs, start=(c == 0),
                                     stop=(c == FC - 1))
                h1_ps.append(hp)
            for hh in range(2):
                if bf1t is None:
                    nc.vector.tensor_scalar_max(h1r[:, hh, :], h1_ps[hh][:],
                                                0.0)
                else:
                    nc.vector.tensor_scalar(h1r[:, hh, :], h1_ps[hh][:],
                                            bf1t[:, hh:hh + 1], 0.0,
                                            ALU.add, ALU.max)
            out_ps = psB.tile([L, BL], f32, tag="mm", name="out_ps")
            for hh in range(2):
                nc.tensor.matmul(out_ps[:],
                                 ctile[:, C_WF2 + hh * L:C_WF2 + (hh + 1) * L],
                                 h1r[:, hh, :], start=(hh == 0),
                                 stop=(hh == 1))
            outs = cpool.tile([L, BL], f32)
            if bf2t is None:
                nc.vector.tensor_copy(outs[:], out_ps[:])
            else:
                nc.vector.tensor_scalar_add(outs[:], out_ps[:], bf2t[:])
            nc.sync.dma_start(out_d[:], outs[:])

    _split_multi_waits(nc)
    return nc


def _prepare_in_maps(inputs):
    lh = np.ascontiguousarray(np.asarray(inputs["last_hidden"], dtype=np.float32))
    submap = np.asarray(inputs["submap"]).astype(np.int64)
    edge_index = np.asarray(inputs["edge_index"]).astype(np.int64)
    assert lh.shape == (B, S, H)
    assert int(inputs.get("num_nodes", N)) == N

    wr = np.asarray(inputs["wr"], dtype=np.float32)
    br = float(np.asarray(inputs["br"], dtype=np.float32))
    W1 = np.asarray(inputs["W1"], dtype=np.float32)
    b1 = np.asarray(inputs["b1"], dtype=np.float32)
    W2 = np.asarray(inputs["W2"], dtype=np.float32)
    b2 = np.asarray(inputs["b2"], dtype=np.float32)
    Wf1 = np.asarray(inputs["Wf1"], dtype=np.float32)
    bf1 = np.asarray(inputs["bf1"], dtype=np.float32)
    Wf2 = np.asarray(inputs["Wf2"], dtype=np.float32)
    bf2 = np.asarray(inputs["bf2"], dtype=np.float32)

    # ---- host-side index prep: adjacency, degrees, counts ----
    src = edge_index[:, 0, :]
    dst = edge_index[:, 1, :]
    flat = (np.arange(B, dtype=np.int64)[:, None] * (N * N) + src * N + dst)
    A = np.bincount(flat.reshape(-1), minlength=B * N * N).astype(np.float32)
    A = A.reshape(B, N, N) + np.eye(N, dtype=np.float32)[None]
    deg = A.sum(axis=1)                      # in-degree incl self-loops
    dinv = 1.0 / np.sqrt(deg)
    ahat = A * dinv[:, :, None] * dinv[:, None, :]

    cflat = np.arange(B, dtype=np.int64)[:, None] * N + submap
    cnt = np.bincount(cflat.reshape(-1), minlength=B * N).astype(np.float32)
    invc = 1.0 / np.maximum(cnt.reshape(B, N), 1.0)

    P = (submap[:, :, None] == np.arange(N)[None, None, :]).astype(np.float32)
    P *= (invc * dinv)[:, None, :]

    # ---- mega-tensor assembly (bf16) ----
    lht = lh.astype(BF16).reshape(B, S, HC, 128).transpose(0, 3, 2, 1)
    p_r = P.astype(BF16).reshape(B, SC, 128, N).transpose(0, 2, 1, 3)
    meg = np.empty((B, 128, MEG_W), dtype=BF16)
    meg[:, :, MEG_LHT:MEG_PG] = lht.reshape(B, 128, HC * S)
    meg[:, :, MEG_PG:MEG_AH] = p_r.reshape(B, 128, SC * N)
    meg[:, :, MEG_AH:MEG_W] = ahat.astype(BF16)

    # ---- consts (bf16), cls block differs per core ----
    consts = np.zeros((128, C_W), dtype=np.float32)
    consts[:, C_W1:C_W1 + HC * GH] = (
        W1.reshape(HC, 128, GH).transpose(1, 0, 2).reshape(128, HC * GH))
    consts[:, C_W2:C_W2 + GH] = W2
    consts[:, C_WF1:C_WF1 + FC * 2 * 128] = (
        Wf1.reshape(FC, 128, 2, 128).transpose(1, 0, 2, 3).reshape(128, -1))
    consts[:, C_WF2:C_WF2 + 2 * L] = (
        Wf2.reshape(2, 128, L).transpose(1, 0, 2).reshape(128, 2 * L))
    consts[:, C_MEAN] = 1.0 / N
    consts[:, C_WR:C_WR + HC] = wr.reshape(HC, 128).T
    consts[:, C_ONES:C_ONES + 128] = 1.0
    consts[:, C_IDENT:C_IDENT + 128] = np.eye(128, dtype=np.float32)

    b1b = np.ascontiguousarray(np.broadcast_to(b1, (128, GH)).astype(np.float32))
    b2b = np.ascontiguousarray(np.broadcast_to(b2, (128, GH)).astype(np.float32))
    bf1b = np.ascontiguousarray(bf1.reshape(2, 128).T.astype(np.float32))
    bf2b = np.ascontiguousarray(bf2.reshape(L, 1).astype(np.float32))

    in_maps = []
    for i in range(NCORES):
        sl = slice(i * BL, (i + 1) * BL)
        ci = consts.copy()
        ci[:, C_CLS:C_CLS + HC * BL] = (
            lh[sl, 0, :].reshape(BL, HC, 128).transpose(2, 1, 0)
            .reshape(128, HC * BL))
        in_maps.append({
            "meg": np.ascontiguousarray(meg[sl]),
            "consts": ci.astype(BF16),
            "b1b": b1b, "b2b": b2b, "bf1b": bf1b, "bf2b": bf2b,
        })
    flags = (br, bool(np.all(b1 == 0)), bool(np.all(b2 == 0)),
             bool(np.all(bf1 == 0)), bool(np.all(bf2 == 0)))
    return in_maps, flags


def _run(inputs, trace=False):
    in_maps, flags = _prepare_in_maps(inputs)
    key = ("prog",) + flags
    if key not in _CACHE:
        _CACHE[key] = build_program(*flags)
    nc = _CACHE[key]
    res = run_bass_kernel_spmd(nc, in_maps, list(range(NCORES)), trace=trace)
    out = np.concatenate(
        [np.asarray(res.results[i]["out"]).T for i in range(NCORES)],
        axis=0).astype(np.float32)
    return out, res


def kernel(**inputs) -> np.ndarray:
    out, _ = _run(inputs, trace=False)
    return out


# revision 15
# speedup vs baseline: 2.1135x; 1.0709x over previous
"""Trainium2 Bass kernel for BioBERT-ARG-GNN (gated pooling + 2-layer GCN + MLP head).

Strategy (v4): pure data parallel over batch B=64 across 8 NeuronCores (8
graphs per core).  Host precomputes index-derived structures (one-hot
pooling matrix P' with 1/cnt and D^-1/2 folded in, normalized adjacency
\hat A = D^-1/2 (A+I) D^-1/2) and ships them bf16 together with a bf16
TRANSPOSED copy of last_hidden (lhT, [hidden, tokens]) in ONE mega-tensor
per graph.  The transposed layout lets BOTH the gate logits (wr . lh_t)
and the W1 projection run on the PE with the contraction over hidden:

    yT[gh, t]  = sum_hc W1c^T @ lhT_c          (6 matmuls, free=512)
    lg[0:1, t] = sum_hc wr_c^T @ lhT_c         (6 matmuls, free=512)

sigmoid(lg) -> gate row [1, 512]; a 1-row matmul against a ones vector
broadcasts it to [128, 512]; DVE multiplies it into yT while casting to
bf16 (ygT); ONE DMA-XBAR transpose per graph turns ygT into token-major
y chunks; pooling then contracts tokens directly: t1 = P'^T (g*y) =
(pool(gated lh)) @ W1 — no nf materialization, no PE transposes.  GCN
layers use \hat A as stationary; FC head is batched over all 8 graphs
with no transposes.  Phase A (projection matmuls) streams back-to-back
paced by the 8 graph DMAs (SP + GPSIMD SWDGE rings alternate), keeping
the PE continuously busy; phase B runs pooling + GCN in per-stage rounds
across graphs so every PE op's cross-engine inputs are a full round old.
"""

import os
import sys

import numpy as np

for _p in ("/opt/trn_rl_repo", "/root/.axon_site/_ro/trn_rl_repo"):
    if os.path.isdir(_p) and _p not in sys.path:
        sys.path.insert(0, _p)

import ml_dtypes  # noqa: E402
import concourse.bass as bass  # noqa: E402
import concourse.mybir as mybir  # noqa: E402
from concourse import tile  # noqa: E402
from concourse.bass_utils import run_bass_kernel_spmd  # noqa: E402

# Problem shapes (hardcoded per contest rules).
B, S, H = 64, 512, 768
N, E = 128, 1024
GH, FH, L = 128, 256, 2
NCORES = 8
BL = B // NCORES  # graphs per core
SC = S // 128     # subtoken chunks per graph
HC = H // 128     # BERT-hidden chunks
FC = (H + GH) // 128  # concat-feature chunks for the FC head

# mega-tensor column offsets (bf16)
MEG_LHT = 0             # [HC*S] = 3072: lhT[p, hc*S + t] = lh[t, hc*128+p]
MEG_PG = HC * S         # [SC*N] = 512: P' (one-hot * invc * dinv), token-major
MEG_AH = MEG_PG + SC * N  # [N] = 128: \hat A row block
MEG_W = MEG_AH + N      # 3712 total

# consts column offsets (bf16)
C_W1 = 0                      # [HC*GH] = 768: [p, hc*128+j] = W1[hc*128+p, j]
C_W2 = C_W1 + HC * GH         # [GH]
C_WF1 = C_W2 + GH             # [FC*2*128] = 1792
C_WF2 = C_WF1 + FC * 2 * 128  # [2*L] = 4
C_CLS = C_WF2 + 2 * L         # [HC*BL] = 48
C_MEAN = C_CLS + HC * BL      # [1]
C_WR = C_MEAN + 12            # [HC] = 6: [p, hc] = wr[hc*128+p]
C_ONES = C_WR + HC + 10       # [128] of 1.0 (row 0 used as [1,128] lhsT)
C_IDENT = C_ONES + 128        # [128]
C_W = C_IDENT + 128

f32 = mybir.dt.float32
bf16 = mybir.dt.bfloat16
AFT = mybir.ActivationFunctionType
ALU = mybir.AluOpType
BF16 = ml_dtypes.bfloat16

_CACHE = {}


def _split_multi_waits(nc: bass.Bass) -> int:
    """Walrus in this container accepts one sync-wait per instruction; split
    extra waits into single-wait EventSemaphore nops just before it."""
    n_split = 0
    for fn in nc.m.functions:
        for blk in fn.blocks:
            new_instrs = []
            changed = False
            for inst in blk.instructions:
                si = getattr(inst, "sync_info", None)
                if si is not None and si.on_wait is not None and len(si.on_wait) > 1:
                    waits = list(si.on_wait)
                    for j, w in enumerate(waits[:-1]):
                        ev = mybir.InstEventSemaphore(
                            name=f"{inst.name}_ws{j}",
                            ins=[], outs=[],
                            engine=inst.engine,
                            sync_info=mybir.SyncInfo(on_wait=[w], on_update=[]),
                        )
                        new_instrs.append(ev)
                    inst.sync_info = mybir.SyncInfo(
                        on_wait=[waits[-1]], on_update=list(si.on_update))
                    n_split += 1
                    changed = True
                new_instrs.append(inst)
            if changed:
                blk.instructions = new_instrs
    return n_split


def build_program(br_val: float, b1_zero: bool, b2_zero: bool,
                  bf1_zero: bool, bf2_zero: bool) -> bass.Bass:
    nc = bass.Bass()

    meg_d = nc.declare_dram_parameter("meg", [BL, 128, MEG_W], bf16, isOutput=False)
    consts_d = nc.declare_dram_parameter("consts", [128, C_W], bf16, isOutput=False)
    b1b_d = nc.declare_dram_parameter("b1b", [128, GH], f32, isOutput=False)
    b2b_d = nc.declare_dram_parameter("b2b", [128, GH], f32, isOutput=False)
    bf1b_d = nc.declare_dram_parameter("bf1b", [128, 2], f32, isOutput=False)
    bf2b_d = nc.declare_dram_parameter("bf2b", [L, 1], f32, isOutput=False)
    out_d = nc.declare_dram_parameter("out", [L, BL], f32, isOutput=True)

    with tile.TileContext(nc) as tc:
        with (
            tc.tile_pool(name="const", bufs=1) as cpool,
            tc.tile_pool(name="megp", bufs=BL) as megpool,
            tc.tile_pool(name="work", bufs=3) as wpool,
            tc.tile_pool(name="psY", bufs=2, space="PSUM") as psY,
            tc.tile_pool(name="psG", bufs=1, space="PSUM") as psG,
            tc.tile_pool(name="psL", bufs=2, space="PSUM") as psL,
            tc.tile_pool(name="psB", bufs=3, space="PSUM") as psB,
        ):
            ctile = cpool.tile([128, C_W], bf16)
            nc.scalar.dma_start(ctile[:], consts_d[:])
            b1t = b2t = bf1t = bf2t = None
            if not b1_zero:
                b1t = cpool.tile([128, GH], f32, name="b1t")
                nc.scalar.dma_start(b1t[:], b1b_d[:])
            if not b2_zero:
                b2t = cpool.tile([128, GH], f32, name="b2t")
                nc.scalar.dma_start(b2t[:], b2b_d[:])
            if not bf1_zero:
                bf1t = cpool.tile([128, 2], f32, name="bf1t")
                nc.scalar.dma_start(bf1t[:], bf1b_d[:])
            if not bf2_zero:
                bf2t = cpool.tile([L, 1], f32, name="bf2t")
                nc.scalar.dma_start(bf2t[:], bf2b_d[:])
            catT6 = cpool.tile([128, BL], bf16)
            h1r = cpool.tile([128, 2, BL], bf16)

            # meg delivery: singles early (latency), pairs late (fewer DGE
            # gaps); sync HWDGE ring and gpsimd SWDGE ring alternate.
            megs = [None] * BL
            m0 = megpool.tile([128, MEG_W], bf16, tag="m0", bufs=1, name="m0")
            nc.sync.dma_start(m0[:], meg_d[0])
            megs[0] = m0
            p13 = megpool.tile([128, 2, MEG_W], bf16, tag="p13", bufs=1,
                               name="p13")
            nc.gpsimd.dma_start(p13[:], meg_d[1:4:2].rearrange("g p w -> p g w"))
            megs[1], megs[3] = p13[:, 0, :], p13[:, 1, :]
            m2 = megpool.tile([128, MEG_W], bf16, tag="m2", bufs=1, name="m2")
            nc.sync.dma_start(m2[:], meg_d[2])
            megs[2] = m2
            p46 = megpool.tile([128, 2, MEG_W], bf16, tag="p46", bufs=1,
                               name="p46")
            nc.sync.dma_start(p46[:], meg_d[4:7:2].rearrange("g p w -> p g w"))
            megs[4], megs[6] = p46[:, 0, :], p46[:, 1, :]
            p57 = megpool.tile([128, 2, MEG_W], bf16, tag="p57", bufs=1,
                               name="p57")
            nc.gpsimd.dma_start(p57[:], meg_d[5:8:2].rearrange("g p w -> p g w"))
            megs[5], megs[7] = p57[:, 0, :], p57[:, 1, :]

            W2c = ctile[:, C_W2:C_W2 + GH]
            MEAN = ctile[:, C_MEAN:C_MEAN + 1]
            IDENT = ctile[:, C_IDENT:C_IDENT + 128]
            ONES1 = ctile[0:1, C_ONES:C_ONES + 128]

            yT_ps = [None] * BL
            lg_ps = [None] * BL
            gate_sb = [None] * BL
            y_sb = [None] * BL
            t1sb = [None] * BL
            x1 = [None] * BL
            x1t = [None] * BL
            t2sb = [None] * BL
            x2 = [None] * BL

            def relu_to(out_sb, z_ps, bias_tile, tag):
                if bias_tile is None:
                    nc.vector.tensor_scalar_max(out_sb[:], z_ps[:], 0.0)
                else:
                    tmp = wpool.tile([128, GH], f32, tag=tag + "b", bufs=2,
                                     name=tag + "b")
                    nc.vector.tensor_tensor(tmp[:], z_ps[:], bias_tile[:],
                                            ALU.add)
                    nc.vector.tensor_scalar_max(out_sb[:], tmp[:], 0.0)

            ygsb2 = [None] * (BL // 2)
            y2 = [None] * (BL // 2)

            def gate_into_y(g):
                """sigmoid -> broadcast -> gate into yT; XBAR per pair."""
                k, half = g // 2, g % 2
                gate_sb[g] = wpool.tile([1, S], bf16, tag="gt", bufs=2,
                                        name="gate_sb")
                nc.scalar.activation(gate_sb[g][:], lg_ps[g][:], AFT.Sigmoid,
                                     bias=float(br_val))
                gb_ps = psG.tile([128, S], f32, tag="gb", name="gb_ps")
                nc.tensor.matmul(gb_ps[:], ONES1, gate_sb[g][:],
                                 start=True, stop=True)
                gb_sb = wpool.tile([128, S], bf16, tag="gbsb", bufs=2,
                                   name="gb_sb")
                nc.vector.tensor_copy(gb_sb[:], gb_ps[:])
                if half == 0:
                    ygsb2[k] = wpool.tile([128, 2, S], bf16, tag="ygsb",
                                          bufs=2, name="ygsb")
                nc.vector.tensor_tensor(ygsb2[k][:, half, :], yT_ps[g][:],
                                        gb_sb[:], ALU.mult)
                if half == 1:
                    y2[k] = wpool.tile([128, 2 * SC, 128], bf16, tag="ysb",
                                       bufs=BL // 2, name="y_sb")
                    nc.scalar.dma_start(y2[k][:], ygsb2[k][:], transpose=True)
                    y_sb[2 * k] = y2[k][:, 0:SC, :]
                    y_sb[2 * k + 1] = y2[k][:, SC:2 * SC, :]

            def pool_g(g):
                t1_ps = psB.tile([128, GH], f32, tag="mm", name="t1_ps")
                for c in range(SC):
                    nc.tensor.matmul(
                        t1_ps[:],
                        megs[g][:, MEG_PG + c * N:MEG_PG + (c + 1) * N],
                        y_sb[g][:, c, :], start=(c == 0), stop=(c == SC - 1))
                t1sb[g] = wpool.tile([128, GH], bf16, tag="t1sb", bufs=BL,
                                     name="t1sb")
                nc.scalar.copy(t1sb[g][:], t1_ps[:])

            # ---- phase A: projection + gate + pooling, DMA-paced ----
            for s in range(BL):
                yT_ps[s] = psY.tile([128, S], f32, tag="yt", name="yT_ps")
                lg_ps[s] = psL.tile([1, S], f32, tag="lg", name="lg_ps")
                for hc in range(HC):
                    lht_c = megs[s][:, MEG_LHT + hc * S:MEG_LHT + (hc + 1) * S]
                    nc.tensor.matmul(
                        yT_ps[s][:],
                        ctile[:, C_W1 + hc * GH:C_W1 + (hc + 1) * GH],
                        lht_c, start=(hc == 0), stop=(hc == HC - 1))
                    nc.tensor.matmul(
                        lg_ps[s][:], ctile[:, C_WR + hc:C_WR + hc + 1],
                        lht_c, start=(hc == 0), stop=(hc == HC - 1))
                if s >= 1:
                    gate_into_y(s - 1)
                if s >= 3 and s % 2 == 1:
                    k = (s - 3) // 2
                    pool_g(2 * k)
                    pool_g(2 * k + 1)
            gate_into_y(BL - 1)
            pool_g(BL - 2)
            pool_g(BL - 1)

            # ---- phase B: GCN in rounds across graphs ----
            for g in range(BL):
                z_ps = psB.tile([128, GH], f32, tag="mm", name="z_ps")
                nc.tensor.matmul(z_ps[:], megs[g][:, MEG_AH:MEG_AH + N],
                                 t1sb[g][:], start=True, stop=True)
                x1[g] = wpool.tile([128, GH], bf16, tag="x1", bufs=BL,
                                   name="x1")
                relu_to(x1[g], z_ps, b1t, "x1")
            for g in range(BL):
                xt_ps = psB.tile([128, GH], bf16, tag="mm", name="xt_ps")
                nc.tensor.transpose(xt_ps[:], x1[g][:], IDENT)
                x1t[g] = wpool.tile([128, GH], bf16, tag="x1t", bufs=BL,
                                    name="x1t")
                nc.vector.tensor_copy(x1t[g][:], xt_ps[:])
            for g in range(BL):
                t2_ps = psB.tile([128, GH], f32, tag="mm", name="t2_ps")
                nc.tensor.matmul(t2_ps[:], x1t[g][:], W2c,
                                 start=True, stop=True)
                t2sb[g] = wpool.tile([128, GH], bf16, tag="t2sb", bufs=BL,
                                     name="t2sb")
                nc.scalar.copy(t2sb[g][:], t2_ps[:])
            for g in range(BL):
                z2_ps = psB.tile([128, GH], f32, tag="mm", name="z2_ps")
                nc.tensor.matmul(z2_ps[:], megs[g][:, MEG_AH:MEG_AH + N],
                                 t2sb[g][:], start=True, stop=True)
                x2[g] = wpool.tile([128, GH], bf16, tag="x2", bufs=BL,
                                   name="x2")
                relu_to(x2[g], z2_ps, b2t, "x2")
            for g in range(BL):
                mp_ps = psB.tile([128, 1], f32, tag="mm", name="mp_ps")
                nc.tensor.matmul(mp_ps[:], x2[g][:], MEAN,
                                 start=True, stop=True)
                nc.vector.tensor_copy(catT6[:, g:g + 1], mp_ps[:])

            # ---------- FC head over all BL graphs ----------
            h1_ps = []
            for hh in range(2):
                hp = psB.tile([128, BL], f32, tag="mm", name=f"h1_ps{hh}")
                for c in range(FC):
                    lhsT = ctile[:, C_WF1 + (c * 2 + hh) * 128:
                                 C_WF1 + (c * 2 + hh + 1) * 128]
                    rhs = (ctile[:, C_CLS + c * BL:C_CLS + (c + 1) * BL]
                           if c < HC else catT6[:])
                    nc.tensor.matmul(hp[:], lhsT, rhs, start=(c == 0),
                                     stop=(c == FC - 1))
                h1_ps.append(hp)
            for hh in range(2):
                if bf1t is None:
                    nc.vector.tensor_scalar_max(h1r[:, hh, :], h1_ps[hh][:],
                                                0.0)
                else:
                    nc.vector.tensor_scalar(h1r[:, hh, :], h1_ps[hh][:],
                                            bf1t[:, hh:hh + 1], 0.0,
                                            ALU.add, ALU.max)
            out_ps = psB.tile([L, BL], f32, tag="mm", name="out_ps")
            for hh in range(2):
                nc.tensor.matmul(out_ps[:],
                                 ctile[:, C_WF2 + hh * L:C_WF2 + (hh + 1) * L],
                                 h1r[:, hh, :], start=(hh == 0),
                                 stop=(hh == 1))
            outs = cpool.tile([L, BL], f32)
            if bf2t is None:
                nc.vector.tensor_copy(outs[:], out_ps[:])
            else:
                nc.vector.tensor_scalar_add(outs[:], out_ps[:], bf2t[:])
            nc.sync.dma_start(out_d[:], outs[:])

    _split_multi_waits(nc)
    return nc


def _prepare_in_maps(inputs):
    lh = np.ascontiguousarray(np.asarray(inputs["last_hidden"], dtype=np.float32))
    submap = np.asarray(inputs["submap"]).astype(np.int64)
    edge_index = np.asarray(inputs["edge_index"]).astype(np.int64)
    assert lh.shape == (B, S, H)
    assert int(inputs.get("num_nodes", N)) == N

    wr = np.asarray(inputs["wr"], dtype=np.float32)
    br = float(np.asarray(inputs["br"], dtype=np.float32))
    W1 = np.asarray(inputs["W1"], dtype=np.float32)
    b1 = np.asarray(inputs["b1"], dtype=np.float32)
    W2 = np.asarray(inputs["W2"], dtype=np.float32)
    b2 = np.asarray(inputs["b2"], dtype=np.float32)
    Wf1 = np.asarray(inputs["Wf1"], dtype=np.float32)
    bf1 = np.asarray(inputs["bf1"], dtype=np.float32)
    Wf2 = np.asarray(inputs["Wf2"], dtype=np.float32)
    bf2 = np.asarray(inputs["bf2"], dtype=np.float32)

    # ---- host-side index prep: adjacency, degrees, counts ----
    src = edge_index[:, 0, :]
    dst = edge_index[:, 1, :]
    flat = (np.arange(B, dtype=np.int64)[:, None] * (N * N) + src * N + dst)
    A = np.bincount(flat.reshape(-1), minlength=B * N * N).astype(np.float32)
    A = A.reshape(B, N, N) + np.eye(N, dtype=np.float32)[None]
    deg = A.sum(axis=1)                      # in-degree incl self-loops
    dinv = 1.0 / np.sqrt(deg)
    ahat = A * dinv[:, :, None] * dinv[:, None, :]

    cflat = np.arange(B, dtype=np.int64)[:, None] * N + submap
    cnt = np.bincount(cflat.reshape(-1), minlength=B * N).astype(np.float32)
    invc = 1.0 / np.maximum(cnt.reshape(B, N), 1.0)

    P = (submap[:, :, None] == np.arange(N)[None, None, :]).astype(np.float32)
    P *= (invc * dinv)[:, None, :]

    # ---- mega-tensor assembly (bf16) ----
    lht = lh.astype(BF16).reshape(B, S, HC, 128).transpose(0, 3, 2, 1)
    p_r = P.astype(BF16).reshape(B, SC, 128, N).transpose(0, 2, 1, 3)
    meg = np.empty((B, 128, MEG_W), dtype=BF16)
    meg[:, :, MEG_LHT:MEG_PG] = lht.reshape(B, 128, HC * S)
    meg[:, :, MEG_PG:MEG_AH] = p_r.reshape(B, 128, SC * N)
    meg[:, :, MEG_AH:MEG_W] = ahat.astype(BF16)

    # ---- consts (bf16), cls block differs per core ----
    consts = np.zeros((128, C_W), dtype=np.float32)
    consts[:, C_W1:C_W1 + HC * GH] = (
        W1.reshape(HC, 128, GH).transpose(1, 0, 2).reshape(128, HC * GH))
    consts[:, C_W2:C_W2 + GH] = W2
    consts[:, C_WF1:C_WF1 + FC * 2 * 128] = (
        Wf1.reshape(FC, 128, 2, 128).transpose(1, 0, 2, 3).reshape(128, -1))
    consts[:, C_WF2:C_WF2 + 2 * L] = (
        Wf2.reshape(2, 128, L).transpose(1, 0, 2).reshape(128, 2 * L))
    consts[:, C_MEAN] = 1.0 / N
    consts[:, C_WR:C_WR + HC] = wr.reshape(HC, 128).T
    consts[:, C_ONES:C_ONES + 128] = 1.0
    consts[:, C_IDENT:C_IDENT + 128] = np.eye(128, dtype=np.float32)

    b1b = np.ascontiguousarray(np.broadcast_to(b1, (128, GH)).astype(np.float32))
    b2b = np.ascontiguousarray(np.broadcast_to(b2, (128, GH)).astype(np.float32))
    bf1b = np.ascontiguousarray(bf1.reshape(2, 128).T.astype(np.float32))
    bf2b = np.ascontiguousarray(bf2.reshape(L, 1).astype(np.float32))

    in_maps = []
    for i in range(NCORES):
        sl = slice(i * BL, (i + 1) * BL)
        ci = consts.copy()
        ci[:, C_CLS:C_CLS + HC * BL] = (
            lh[sl, 0, :].reshape(BL, HC, 128).transpose(2, 1, 0)
            .reshape(128, HC * BL))
        in_maps.append({
            "meg": np.ascontiguousarray(meg[sl]),
            "consts": ci.astype(BF16),
            "b1b": b1b, "b2b": b2b, "bf1b": bf1b, "bf2b": bf2b,
        })
    flags = (br, bool(np.all(b1 == 0)), bool(np.all(b2 == 0)),
             bool(np.all(bf1 == 0)), bool(np.all(bf2 == 0)))
    return in_maps, flags


def _run(inputs, trace=False):
    in_maps, flags = _prepare_in_maps(inputs)
    key = ("prog",) + flags
    if key not in _CACHE:
        _CACHE[key] = build_program(*flags)
    nc = _CACHE[key]
    res = run_bass_kernel_spmd(nc, in_maps, list(range(NCORES)), trace=trace)
    out = np.concatenate(
        [np.asarray(res.results[i]["out"]).T for i in range(NCORES)],
        axis=0).astype(np.float32)
    return out, res


def kernel(**inputs) -> np.ndarray:
    out, _ = _run(inputs, trace=False)
    return out


# revision 21
# speedup vs baseline: 2.2291x; 1.0547x over previous
"""Trainium2 Bass kernel for BioBERT-ARG-GNN (gated pooling + 2-layer GCN + MLP head).

Strategy (v4): pure data parallel over batch B=64 across 8 NeuronCores (8
graphs per core).  Host precomputes index-derived structures (one-hot
pooling matrix P' with 1/cnt and D^-1/2 folded in, normalized adjacency
\hat A = D^-1/2 (A+I) D^-1/2) and ships them bf16 together with a bf16
TRANSPOSED copy of last_hidden (lhT, [hidden, tokens]) in ONE mega-tensor
per graph.  The transposed layout lets BOTH the gate logits (wr . lh_t)
and the W1 projection run on the PE with the contraction over hidden:

    yT[gh, t]  = sum_hc W1c^T @ lhT_c          (6 matmuls, free=512)
    lg[0:1, t] = sum_hc wr_c^T @ lhT_c         (6 matmuls, free=512)

sigmoid(lg) -> gate row [1, 512]; a 1-row matmul against a ones vector
broadcasts it to [128, 512]; DVE multiplies it into yT while casting to
bf16 (ygT); ONE DMA-XBAR transpose per graph turns ygT into token-major
y chunks; pooling then contracts tokens directly: t1 = P'^T (g*y) =
(pool(gated lh)) @ W1 — no nf materialization, no PE transposes.  GCN
layers use \hat A as stationary; FC head is batched over all 8 graphs
with no transposes.  Phase A (projection matmuls) streams back-to-back
paced by the 8 graph DMAs (SP + GPSIMD SWDGE rings alternate), keeping
the PE continuously busy; phase B runs pooling + GCN in per-stage rounds
across graphs so every PE op's cross-engine inputs are a full round old.
"""

import os
import sys

import numpy as np

for _p in ("/opt/trn_rl_repo", "/root/.axon_site/_ro/trn_rl_repo"):
    if os.path.isdir(_p) and _p not in sys.path:
        sys.path.insert(0, _p)

import ml_dtypes  # noqa: E402
import concourse.bass as bass  # noqa: E402
import concourse.mybir as mybir  # noqa: E402
from concourse import tile  # noqa: E402
from concourse.bass_utils import run_bass_kernel_spmd  # noqa: E402

# Problem shapes (hardcoded per contest rules).
B, S, H = 64, 512, 768
N, E = 128, 1024
GH, FH, L = 128, 256, 2
NCORES = 8
BL = B // NCORES  # graphs per core
SC = S // 128     # subtoken chunks per graph
HC = H // 128     # BERT-hidden chunks
FC = (H + GH) // 128  # concat-feature chunks for the FC head

# mega-tensor column offsets (bf16)
MEG_LHT = 0             # [HC*S] = 3072: lhT[p, hc*S + t] = lh[t, hc*128+p]
MEG_PG = HC * S         # [SC*N] = 512: P' (one-hot * invc * dinv), token-major
MEG_AH = MEG_PG + SC * N  # [N] = 128: \hat A row block
MEG_W = MEG_AH + N      # 3712 total

# consts column offsets (bf16)
C_W1 = 0                      # [HC*GH] = 768: [p, hc*128+j] = W1[hc*128+p, j]
C_W2 = C_W1 + HC * GH         # [GH]
C_WF1 = C_W2 + GH             # [FC*2*128] = 1792
C_WF2 = C_WF1 + FC * 2 * 128  # [2*L] = 4
C_CLS = C_WF2 + 2 * L         # [HC*BL] = 48
C_MEAN = C_CLS + HC * BL      # [1]
C_WRB = C_MEAN + 12           # [HC*128] = 768: [p, hc*128+m] = wr[hc*128+p]
C_IDENT = C_WRB + HC * 128    # [128]
C_W = C_IDENT + 128

f32 = mybir.dt.float32
bf16 = mybir.dt.bfloat16
AFT = mybir.ActivationFunctionType
ALU = mybir.AluOpType
BF16 = ml_dtypes.bfloat16

_CACHE = {}


def _split_multi_waits(nc: bass.Bass) -> int:
    """Walrus in this container accepts one sync-wait per instruction; split
    extra waits into single-wait EventSemaphore nops just before it."""
    n_split = 0
    for fn in nc.m.functions:
        for blk in fn.blocks:
            new_instrs = []
            changed = False
            for inst in blk.instructions:
                si = getattr(inst, "sync_info", None)
                if si is not None and si.on_wait is not None and len(si.on_wait) > 1:
                    waits = list(si.on_wait)
                    for j, w in enumerate(waits[:-1]):
                        ev = mybir.InstEventSemaphore(
                            name=f"{inst.name}_ws{j}",
                            ins=[], outs=[],
                            engine=inst.engine,
                            sync_info=mybir.SyncInfo(on_wait=[w], on_update=[]),
                        )
                        new_instrs.append(ev)
                    inst.sync_info = mybir.SyncInfo(
                        on_wait=[waits[-1]], on_update=list(si.on_update))
                    n_split += 1
                    changed = True
                new_instrs.append(inst)
            if changed:
                blk.instructions = new_instrs
    return n_split


def build_program(br_val: float, b1_zero: bool, b2_zero: bool,
                  bf1_zero: bool, bf2_zero: bool) -> bass.Bass:
    nc = bass.Bass()

    meg_d = nc.declare_dram_parameter("meg", [BL, 128, MEG_W], bf16, isOutput=False)
    consts_d = nc.declare_dram_parameter("consts", [128, C_W], bf16, isOutput=False)
    b1b_d = nc.declare_dram_parameter("b1b", [128, GH], f32, isOutput=False)
    b2b_d = nc.declare_dram_parameter("b2b", [128, GH], f32, isOutput=False)
    bf1b_d = nc.declare_dram_parameter("bf1b", [128, 2], f32, isOutput=False)
    bf2b_d = nc.declare_dram_parameter("bf2b", [L, 1], f32, isOutput=False)
    out_d = nc.declare_dram_parameter("out", [L, BL], f32, isOutput=True)

    with tile.TileContext(nc) as tc:
        with (
            tc.tile_pool(name="const", bufs=1) as cpool,
            tc.tile_pool(name="megp", bufs=BL) as megpool,
            tc.tile_pool(name="work", bufs=3) as wpool,
            tc.tile_pool(name="psY", bufs=2, space="PSUM") as psY,
            tc.tile_pool(name="psL", bufs=2, space="PSUM") as psL,
            tc.tile_pool(name="psB", bufs=4, space="PSUM") as psB,
        ):
            ctile = cpool.tile([128, C_W], bf16)
            nc.scalar.dma_start(ctile[:], consts_d[:])
            b1t = b2t = bf1t = bf2t = None
            if not b1_zero:
                b1t = cpool.tile([128, GH], f32, name="b1t")
                nc.scalar.dma_start(b1t[:], b1b_d[:])
            if not b2_zero:
                b2t = cpool.tile([128, GH], f32, name="b2t")
                nc.scalar.dma_start(b2t[:], b2b_d[:])
            if not bf1_zero:
                bf1t = cpool.tile([128, 2], f32, name="bf1t")
                nc.scalar.dma_start(bf1t[:], bf1b_d[:])
            if not bf2_zero:
                bf2t = cpool.tile([L, 1], f32, name="bf2t")
                nc.scalar.dma_start(bf2t[:], bf2b_d[:])
            catT6 = cpool.tile([128, BL], bf16)
            h1r = cpool.tile([128, 2, BL], bf16)

            # meg delivery: singles early (latency), pairs late (fewer DGE
            # gaps); sync HWDGE ring and gpsimd SWDGE ring alternate.
            megs = [None] * BL
            m0 = megpool.tile([128, MEG_W], bf16, tag="m0", bufs=1, name="m0")
            nc.sync.dma_start(m0[:], meg_d[0])
            megs[0] = m0
            p13 = megpool.tile([128, 2, MEG_W], bf16, tag="p13", bufs=1,
                               name="p13")
            nc.gpsimd.dma_start(p13[:], meg_d[1:4:2].rearrange("g p w -> p g w"))
            megs[1], megs[3] = p13[:, 0, :], p13[:, 1, :]
            m2 = megpool.tile([128, MEG_W], bf16, tag="m2", bufs=1, name="m2")
            nc.sync.dma_start(m2[:], meg_d[2])
            megs[2] = m2
            p46 = megpool.tile([128, 2, MEG_W], bf16, tag="p46", bufs=1,
                               name="p46")
            nc.sync.dma_start(p46[:], meg_d[4:7:2].rearrange("g p w -> p g w"))
            megs[4], megs[6] = p46[:, 0, :], p46[:, 1, :]
            p57 = megpool.tile([128, 2, MEG_W], bf16, tag="p57", bufs=1,
                               name="p57")
            nc.gpsimd.dma_start(p57[:], meg_d[5:8:2].rearrange("g p w -> p g w"))
            megs[5], megs[7] = p57[:, 0, :], p57[:, 1, :]

            W2c = ctile[:, C_W2:C_W2 + GH]
            MEAN = ctile[:, C_MEAN:C_MEAN + 1]
            IDENT = ctile[:, C_IDENT:C_IDENT + 128]

            yT_ps = [None] * BL
            lg_ps = [None] * BL
            gate_sb = [None] * BL
            y_sb = [None] * BL
            t1sb = [None] * BL
            x1 = [None] * BL
            x1t = [None] * BL
            t2sb = [None] * BL
            x2 = [None] * BL

            def relu_to(out_sb, z_ps, bias_tile, tag):
                if bias_tile is None:
                    nc.vector.tensor_scalar_max(out_sb[:], z_ps[:], 0.0)
                else:
                    tmp = wpool.tile([128, GH], f32, tag=tag + "b", bufs=2,
                                     name=tag + "b")
                    nc.vector.tensor_tensor(tmp[:], z_ps[:], bias_tile[:],
                                            ALU.add)
                    nc.vector.tensor_scalar_max(out_sb[:], tmp[:], 0.0)

            ygsb2 = [None] * (BL // 2)
            y2 = [None] * (BL // 2)

            def gate_into_y(g):
                """sigmoid on broadcast logits -> gate into yT; XBAR per pair."""
                k, half = g // 2, g % 2
                gb_sb = wpool.tile([128, S], bf16, tag="gbsb", bufs=2,
                                   name="gb_sb")
                nc.scalar.activation(gb_sb[:], lg_ps[g][:], AFT.Sigmoid,
                                     bias=float(br_val))
                if half == 0:
                    ygsb2[k] = wpool.tile([128, 2, S], bf16, tag="ygsb",
                                          bufs=2, name="ygsb")
                nc.vector.tensor_tensor(ygsb2[k][:, half, :], yT_ps[g][:],
                                        gb_sb[:], ALU.mult)
                if half == 1:
                    y2[k] = wpool.tile([128, 2 * SC, 128], bf16, tag="ysb",
                                       bufs=BL // 2, name="y_sb")
                    nc.scalar.dma_start(y2[k][:], ygsb2[k][:], transpose=True)
                    y_sb[2 * k] = y2[k][:, 0:SC, :]
                    y_sb[2 * k + 1] = y2[k][:, SC:2 * SC, :]

            def pool_g(g):
                t1_ps = psB.tile([128, GH], f32, tag="mm", name="t1_ps")
                for c in range(SC):
                    nc.tensor.matmul(
                        t1_ps[:],
                        megs[g][:, MEG_PG + c * N:MEG_PG + (c + 1) * N],
                        y_sb[g][:, c, :], start=(c == 0), stop=(c == SC - 1))
                t1sb[g] = wpool.tile([128, GH], bf16, tag="t1sb", bufs=BL,
                                     name="t1sb")
                nc.scalar.copy(t1sb[g][:], t1_ps[:])

            # ---- phase A: projection + gate + pooling, DMA-paced ----
            for s in range(BL):
                yT_ps[s] = psY.tile([128, S], f32, tag="yt", name="yT_ps")
                lg_ps[s] = psL.tile([128, S], f32, tag="lg", name="lg_ps")
                for hc in range(HC):
                    lht_c = megs[s][:, MEG_LHT + hc * S:MEG_LHT + (hc + 1) * S]
                    nc.tensor.matmul(
                        yT_ps[s][:],
                        ctile[:, C_W1 + hc * GH:C_W1 + (hc + 1) * GH],
                        lht_c, start=(hc == 0), stop=(hc == HC - 1))
                    nc.tensor.matmul(
                        lg_ps[s][:],
                        ctile[:, C_WRB + hc * 128:C_WRB + (hc + 1) * 128],
                        lht_c, start=(hc == 0), stop=(hc == HC - 1))
                if s >= 1:
                    gate_into_y(s - 1)
                if s >= 3 and s % 2 == 1:
                    k = (s - 3) // 2
                    pool_g(2 * k)
                    pool_g(2 * k + 1)
            gate_into_y(BL - 1)
            pool_g(BL - 2)
            pool_g(BL - 1)

            # ---- phase B: GCN in rounds across graphs ----
            for g in range(BL):
                z_ps = psB.tile([128, GH], f32, tag="mm", name="z_ps")
                nc.tensor.matmul(z_ps[:], megs[g][:, MEG_AH:MEG_AH + N],
                                 t1sb[g][:], start=True, stop=True)
                x1[g] = wpool.tile([128, GH], bf16, tag="x1", bufs=BL,
                                   name="x1")
                relu_to(x1[g], z_ps, b1t, "x1")
            for g in range(BL):
                xt_ps = psB.tile([128, GH], bf16, tag="mm", name="xt_ps")
                nc.tensor.transpose(xt_ps[:], x1[g][:], IDENT)
                x1t[g] = wpool.tile([128, GH], bf16, tag="x1t", bufs=BL,
                                    name="x1t")
                nc.vector.tensor_copy(x1t[g][:], xt_ps[:])
            for g in range(BL):
                t2_ps = psB.tile([128, GH], f32, tag="mm", name="t2_ps")
                nc.tensor.matmul(t2_ps[:], x1t[g][:], W2c,
                                 start=True, stop=True)
                t2sb[g] = wpool.tile([128, GH], bf16, tag="t2sb", bufs=BL,
                                     name="t2sb")
                nc.scalar.copy(t2sb[g][:], t2_ps[:])
            for g in range(BL):
                z2_ps = psB.tile([128, GH], f32, tag="mm", name="z2_ps")
                nc.tensor.matmul(z2_ps[:], megs[g][:, MEG_AH:MEG_AH + N],
                                 t2sb[g][:], start=True, stop=True)
                x2[g] = wpool.tile([128, GH], bf16, tag="x2", bufs=BL,
                                   name="x2")
                relu_to(x2[g], z2_ps, b2t, "x2")
            for g in range(BL):
                mp_ps = psB.tile([128, 1], f32, tag="mm", name="mp_ps")
                nc.tensor.matmul(mp_ps[:], x2[g][:], MEAN,
                                 start=True, stop=True)
                nc.vector.tensor_copy(catT6[:, g:g + 1], mp_ps[:])

            # ---------- FC head over all BL graphs ----------
            h1_ps = []
            for hh in range(2):
                hp = psB.tile([128, BL], f32, tag="mm", name=f"h1_ps{hh}")
                for c in range(FC):
                    lhsT = ctile[:, C_WF1 + (c * 2 + hh) * 128:
                                 C_WF1 + (c * 2 + hh + 1) * 128]
                    rhs = (ctile[:, C_CLS + c * BL:C_CLS + (c + 1) * BL]
                           if c < HC else catT6[:])
                    nc.tensor.matmul(hp[:], lhsT, rhs, start=(c == 0),
                                     stop=(c == FC - 1))
                h1_ps.append(hp)
            for hh in range(2):
                if bf1t is None:
                    nc.vector.tensor_scalar_max(h1r[:, hh, :], h1_ps[hh][:],
                                                0.0)
                else:
                    nc.vector.tensor_scalar(h1r[:, hh, :], h1_ps[hh][:],
                                            bf1t[:, hh:hh + 1], 0.0,
                                            ALU.add, ALU.max)
            out_ps = psB.tile([L, BL], f32, tag="mm", name="out_ps")
            for hh in range(2):
                nc.tensor.matmul(out_ps[:],
                                 ctile[:, C_WF2 + hh * L:C_WF2 + (hh + 1) * L],
                                 h1r[:, hh, :], start=(hh == 0),
                                 stop=(hh == 1))
            outs = cpool.tile([L, BL], f32)
            if bf2t is None:
                nc.vector.tensor_copy(outs[:], out_ps[:])
            else:
                nc.vector.tensor_scalar_add(outs[:], out_ps[:], bf2t[:])
            nc.sync.dma_start(out_d[:], outs[:])

    _split_multi_waits(nc)
    return nc


def _prepare_in_maps(inputs):
    lh = np.ascontiguousarray(np.asarray(inputs["last_hidden"], dtype=np.float32))
    submap = np.asarray(inputs["submap"]).astype(np.int64)
    edge_index = np.asarray(inputs["edge_index"]).astype(np.int64)
    assert lh.shape == (B, S, H)
    assert int(inputs.get("num_nodes", N)) == N

    wr = np.asarray(inputs["wr"], dtype=np.float32)
    br = float(np.asarray(inputs["br"], dtype=np.float32))
    W1 = np.asarray(inputs["W1"], dtype=np.float32)
    b1 = np.asarray(inputs["b1"], dtype=np.float32)
    W2 = np.asarray(inputs["W2"], dtype=np.float32)
    b2 = np.asarray(inputs["b2"], dtype=np.float32)
    Wf1 = np.asarray(inputs["Wf1"], dtype=np.float32)
    bf1 = np.asarray(inputs["bf1"], dtype=np.float32)
    Wf2 = np.asarray(inputs["Wf2"], dtype=np.float32)
    bf2 = np.asarray(inputs["bf2"], dtype=np.float32)

    # ---- host-side index prep: adjacency, degrees, counts ----
    src = edge_index[:, 0, :]
    dst = edge_index[:, 1, :]
    flat = (np.arange(B, dtype=np.int64)[:, None] * (N * N) + src * N + dst)
    A = np.bincount(flat.reshape(-1), minlength=B * N * N).astype(np.float32)
    A = A.reshape(B, N, N) + np.eye(N, dtype=np.float32)[None]
    deg = A.sum(axis=1)                      # in-degree incl self-loops
    dinv = 1.0 / np.sqrt(deg)
    ahat = A * dinv[:, :, None] * dinv[:, None, :]

    cflat = np.arange(B, dtype=np.int64)[:, None] * N + submap
    cnt = np.bincount(cflat.reshape(-1), minlength=B * N).astype(np.float32)
    invc = 1.0 / np.maximum(cnt.reshape(B, N), 1.0)

    P = (submap[:, :, None] == np.arange(N)[None, None, :]).astype(np.float32)
    P *= (invc * dinv)[:, None, :]

    # ---- mega-tensor assembly (bf16) ----
    lht = lh.astype(BF16).reshape(B, S, HC, 128).transpose(0, 3, 2, 1)
    p_r = P.astype(BF16).reshape(B, SC, 128, N).transpose(0, 2, 1, 3)
    meg = np.empty((B, 128, MEG_W), dtype=BF16)
    meg[:, :, MEG_LHT:MEG_PG] = lht.reshape(B, 128, HC * S)
    meg[:, :, MEG_PG:MEG_AH] = p_r.reshape(B, 128, SC * N)
    meg[:, :, MEG_AH:MEG_W] = ahat.astype(BF16)

    # ---- consts (bf16), cls block differs per core ----
    consts = np.zeros((128, C_W), dtype=np.float32)
    consts[:, C_W1:C_W1 + HC * GH] = (
        W1.reshape(HC, 128, GH).transpose(1, 0, 2).reshape(128, HC * GH))
    consts[:, C_W2:C_W2 + GH] = W2
    consts[:, C_WF1:C_WF1 + FC * 2 * 128] = (
        Wf1.reshape(FC, 128, 2, 128).transpose(1, 0, 2, 3).reshape(128, -1))
    consts[:, C_WF2:C_WF2 + 2 * L] = (
        Wf2.reshape(2, 128, L).transpose(1, 0, 2).reshape(128, 2 * L))
    consts[:, C_MEAN] = 1.0 / N
    consts[:, C_WRB:C_WRB + HC * 128] = np.repeat(
        wr.reshape(HC, 128).T, 128, axis=1).reshape(128, HC * 128)
    consts[:, C_IDENT:C_IDENT + 128] = np.eye(128, dtype=np.float32)

    b1b = np.ascontiguousarray(np.broadcast_to(b1, (128, GH)).astype(np.float32))
    b2b = np.ascontiguousarray(np.broadcast_to(b2, (128, GH)).astype(np.float32))
    bf1b = np.ascontiguousarray(bf1.reshape(2, 128).T.astype(np.float32))
    bf2b = np.ascontiguousarray(bf2.reshape(L, 1).astype(np.float32))

    in_maps = []
    for i in range(NCORES):
        sl = slice(i * BL, (i + 1) * BL)
        ci = consts.copy()
        ci[:, C_CLS:C_CLS + HC * BL] = (
            lh[sl, 0, :].reshape(BL, HC, 128).transpose(2, 1, 0)
            .reshape(128, HC * BL))
        in_maps.append({
            "meg": np.ascontiguousarray(meg[sl]),
            "consts": ci.astype(BF16),
            "b1b": b1b, "b2b": b2b, "bf1b": bf1b, "bf2b": bf2b,
        })
    flags = (br, bool(np.all(b1 == 0)), bool(np.all(b2 == 0)),
             bool(np.all(bf1 == 0)), bool(np.all(bf2 == 0)))
    return in_maps, flags


def _run(inputs, trace=False):
    in_maps, flags = _prepare_in_maps(inputs)
    key = ("prog",) + flags
    if key not in _CACHE:
        _CACHE[key] = build_program(*flags)
    nc = _CACHE[key]
    res = run_bass_kernel_spmd(nc, in_maps, list(range(NCORES)), trace=trace)
    out = np.concatenate(
        [np.asarray(res.results[i]["out"]).T for i in range(NCORES)],
        axis=0).astype(np.float32)
    return out, res


def kernel(**inputs) -> np.ndarray:
    out, _ = _run(inputs, trace=False)
    return out
